# revision 41
# baseline (speedup 1.0000x reference)
"""Trainium2 Bass kernel for 16-head self-attention (D=1024, S=2048, B=2)
with upper-triangular (j >= i) mask and scale 1/head_dim.

Sharding: batch*head-group parallel over 8 cores. Core c handles batch
c//4, heads [4*(c%4), 4*(c%4)+4). Each core computes Q/K/V projections for
its 256 output dims, attention for its 4 heads, and a partial output
projection (its 256 rows of wo). Host sums the 4 partials per batch.

On-chip layout is transposed end-to-end: QT/KT [dh, seq], scores S^T
[seq_k, seq_q] (stationary=K^T chunk, moving=Q^T), exp on ScalarE
PSUM->SBUF with the 1/64 scale folded in, PV as O'^T = V'^T E^T with a
ones-column appended to V so row 64 of O' is the softmax denominator,
then out^T = wo^T O^T (bf16 partials). The host transposes back.

Perf structure (v2):
- mask applied inside the score PSUM accumulation via an extra
  identity x (-1e30 upper-tri) matmul -> no post-exp mask stage.
- attention iterates key chunks jc DESCENDING so the per-qb O' PSUM
  banks free progressively; each qb block normalizes early with
  reciprocal_approx_fast + a K=1 ones matmul broadcast.
- exp runs on [128, 1024] two-bank PSUM chunks (fewer ACTIVATEs).
- Q/K (m=0) and V projections interleave 1:2 so V's per-matmul
  LDWEIGHTS hides under the 512-wide Q/K matmuls; m=1 projections are
  emitted between attention heads as PE filler (and reuse freed O'
  PSUM slots), keeping the PE dense enough to hold the HAM clock warm.
- batched input DMAs (one per weight tensor, x in 4 column quarters).
"""

import itertools
import sys

sys.path.insert(0, "/opt/trn_rl_repo")

import numpy as np

import concourse.bass as bass
import concourse.mybir as mybir
from concourse import tile
from concourse.bass_utils import run_bass_kernel_spmd

# ---------------------------------------------------------------------------
# Workaround: this walrus build supports only 1 sync wait on the SP CTRL
# (drain) instruction; split the TileContext exit drain's waits across
# sequential drains (same-engine program order makes this equivalent).
_MAX_DRAIN_WAITS = 1


def _patched_drain_and_barrier(self, tick_clock, wait_clock):
    from bass_rust import ScopedClock

    nc = self.nc
    drain_inst = nc.sync.drain()
    wait_clock.add_sem_waits(
        drain_inst.ins, ScopedClock({None: tick_clock.global_clock})
    )
    si = drain_inst.ins.sync_info
    if si is not None and len(si.on_wait) > _MAX_DRAIN_WAITS:
        waits = list(si.on_wait)
        si.on_wait = waits[:_MAX_DRAIN_WAITS]
        rest = waits[_MAX_DRAIN_WAITS:]
        while rest:
            chunk, rest = rest[:_MAX_DRAIN_WAITS], rest[_MAX_DRAIN_WAITS:]
            extra = nc.sync.drain()
            esi = extra.ins.sync_info
            if esi is None:
                extra.ins.sync_info = mybir.SyncInfo(on_wait=chunk, on_update=[])
            else:
                esi.on_wait = chunk
    nc.all_engine_barrier()
    assert self.sems is not None
    popped = nc._tile_sem_poison_stack.pop()
    assert popped is self._sem_poison
    nc.clear_and_free_semaphores(list(self.sems.allocated().values()))
    nc.all_engine_barrier()


tile.TileContext._drain_and_barrier = _patched_drain_and_barrier


def _legalize_waits(nc, max_waits=1):
    """This walrus build accepts at most one sync wait per instruction.
    Hoist extra waits onto preceding NoOps on the same engine (same-engine
    program order preserves the gating semantics)."""
    for blk in nc.main_func.blocks:
        out = []
        for inst in blk.instructions:
            si = inst.sync_info
            if si is not None and len(si.on_wait) > max_waits:
                waits = list(si.on_wait)
                si.on_wait = waits[-max_waits:]
                for w in waits[:-max_waits]:
                    nop = mybir.InstNoOp(
                        name=nc.get_next_instruction_name(), ins=[], outs=[]
                    )
                    nop.engine = inst.engine
                    nop.sync_info = mybir.SyncInfo(on_wait=[w], on_update=[])
                    nc.register_instruction(nop)
                    out.append(nop)
            out.append(inst)
        blk.instructions[:] = out


# ---------------------------------------------------------------------------

B, S, D = 2, 2048, 1024
H, HD = 16, 64
SCALE = 1.0 / HD
NCORES = 8
HPC = 4          # heads per core
DHC = HPC * HD   # 256 head-dims per core
P = 128
KC = D // P      # 8 contraction chunks for projections
SC = S // P      # 16 seq chunks of 128
QB = 512         # seq_q block for PV / O-proj
NQB = S // QB    # 4
EC = 1024        # exp chunk width (2 PSUM banks)

F32 = mybir.dt.float32
F32R = mybir.dt.float32r
BF16 = mybir.dt.bfloat16

_COMPILED = None


def _build_nc():
    nc = bass.Bass("TRN2", target_bir_lowering=False, debug=False,
                   num_devices=NCORES)

    # All big inputs are pre-packed host-side into the exact [128, N]
    # SBUF layouts so every load is a contiguous max-rate 2D DMA.
    # xT packed as [p, (quarter, k, s')]: col = qd*4096 + k*512 + s'
    xT = nc.declare_dram_parameter("xT", [P, KC * S], BF16, isOutput=False)
    wq = nc.declare_dram_parameter("wq", [P, KC * DHC], BF16, isOutput=False)
    wk = nc.declare_dram_parameter("wk", [P, KC * DHC], BF16, isOutput=False)
    wv = nc.declare_dram_parameter("wv", [P, KC * DHC], BF16, isOutput=False)
    wo = nc.declare_dram_parameter("wo", [P, 2 * D], BF16, isOutput=False)
    bq = nc.declare_dram_parameter("bq", [2, P, 1], F32, isOutput=False)
    bk = nc.declare_dram_parameter("bk", [2, P, 1], F32, isOutput=False)
    bv = nc.declare_dram_parameter("bv", [P, DHC], F32, isOutput=False)
    trin = nc.declare_dram_parameter("trin", [P, P], BF16, isOutput=False)
    iden = nc.declare_dram_parameter("iden", [P, P], BF16, isOutput=False)
    sel = nc.declare_dram_parameter("sel", [97, 4 * 64], F32, isOutput=False)
    outT0 = nc.declare_dram_parameter("outT0", [D, S], BF16, isOutput=True)
    outT1 = nc.declare_dram_parameter("outT1", [D, S], BF16, isOutput=True)

    with tile.TileContext(nc) as tc:
        dmaq = [nc.sync, nc.scalar]
        dq = [0]

        def dma(out_ap, in_ap):
            eng = dmaq[dq[0] % len(dmaq)]
            dq[0] += 1
            return eng.dma_start(out_ap, in_ap)

        with (
            tc.tile_pool(name="persist", bufs=1) as pp,
            tc.tile_pool(name="stage", bufs=2) as stage,
            tc.tile_pool(name="epool", bufs=4) as epool,
            tc.tile_pool(name="small", bufs=4) as small,
        ):
            # ---------------- persistent SBUF tensors ----------------
            xb = pp.tile([P, KC * S], BF16, tag="xb")          # 32KB
            wqb = pp.tile([P, KC * DHC], BF16, tag="wqb")      # 4KB
            wkb = pp.tile([P, KC * DHC], BF16, tag="wkb")
            wvb = pp.tile([P, KC * DHC], BF16, tag="wvb")
            wob = pp.tile([P, 2 * D], BF16, tag="wob")
            QT = pp.tile([P, 2 * S], BF16, tag="qt")           # 8KB
            KT = pp.tile([P, 2 * S], BF16, tag="kt")
            # V with a ones column per head: 16 chunks x [h0(64) 1 | ...]
            Vb = pp.tile([P, SC * HPC * 65], BF16, tag="vb")   # 8.3KB
            OT = pp.tile([P, 2 * S], BF16, tag="ot")
            trib = pp.tile([P, P], BF16, tag="trib")
            idb = pp.tile([P, P], BF16, tag="idb")
            bq_sb = pp.tile([P, 2], F32, tag="bq")
            bk_sb = pp.tile([P, 2], F32, tag="bk")
            bv_bc = pp.tile([P, DHC], F32, tag="bvbc")
            # One-hot selector: sel[:, 64r:64r+64] has row 32r all-ones
            # (denominator rows sit at 32-aligned partitions; the K=97
            # matmul broadcasts row 32r of drecr to 64 partitions).
            selb = pp.tile([97, 4 * 64], F32, tag="selb")

            # xb layout [p, (qd, k, s')]: QK group nb reads
            # xq[:, nb, k, :]; V s-chunk reads xq[:, s//4, k, (s%4)*128..]
            xq = xb[:].rearrange("p (q k s) -> p q k s", q=4, k=KC)
            wqv = wqb[:].rearrange("p (k c) -> p k c", k=KC)
            wkv = wkb[:].rearrange("p (k c) -> p k c", k=KC)
            wvv = wvb[:].rearrange("p (k c) -> p k c", k=KC)
            wov = wob[:].rearrange("p (c d) -> p c d", c=2)

            # ---------------- input DMAs (all contiguous 2D) ----------
            nc.sync.dma_start(bq_sb[:, 0:1], bq[0])
            nc.sync.dma_start(bq_sb[:, 1:2], bq[1])
            nc.scalar.dma_start(bk_sb[:, 0:1], bk[0])
            nc.scalar.dma_start(bk_sb[:, 1:2], bk[1])
            # sync ring: x quarters (the critical path); scalar: weights
            for qd in range(4):
                nc.sync.dma_start(xb[:, qd * 4096:(qd + 1) * 4096],
                                  xT[:, qd * 4096:(qd + 1) * 4096])
            nc.scalar.dma_start(wqb[:], wq[:, :])
            nc.scalar.dma_start(wvb[:], wv[:, :])
            nc.scalar.dma_start(wkb[:], wk[:, :])
            nc.scalar.dma_start(bv_bc[:], bv[:, :])
            nc.scalar.dma_start(trib[:], trin[:, :])
            nc.scalar.dma_start(idb[:], iden[:, :])
            nc.scalar.dma_start(wob[:], wo[:, :])
            nc.scalar.dma_start(selb[:], sel[:, :])
            # ones column of V (col 64 of each head's 65-block)
            nc.gpsimd.memset(
                Vb[:].rearrange("p (s h x) -> p s h x", s=SC, h=HPC)
                [:, :, :, 64:65], 1.0)

            # ---------------- helpers ----------------
            def emit_qk_group(aps, dst, wbv, bias, m, nb, tag="pqk"):
                ps = aps.tile([P, QB], F32, tag=tag, name=f"pqk{m}_{nb}")
                yield  # allow interleave before the first MM
                for k in range(KC):
                    nc.tensor.matmul(
                        ps[:],
                        wbv[:, k, m * P:(m + 1) * P],
                        xq[:, nb, k, :],
                        start=(k == 0),
                        stop=(k == KC - 1),
                    )
                    yield
                nc.vector.tensor_scalar_add(
                    dst[:, m * S + nb * QB: m * S + (nb + 1) * QB],
                    ps[:],
                    bias[:, m:m + 1],
                )

            def emit_v_group(aps, s):
                ps = aps.tile([P, DHC], F32, tag="pv")
                yield
                for k in range(KC):
                    nc.tensor.matmul(
                        ps[:],
                        xq[:, s // 4, k, (s % 4) * P:(s % 4 + 1) * P],
                        wvv[:, k, :],
                        start=(k == 0),
                        stop=(k == KC - 1),
                    )
                    yield
                vout = Vb[:, s * 260:(s + 1) * 260].rearrange(
                    "p (h x) -> p h x", h=HPC)[:, :, 0:64]
                psr = ps[:].rearrange("p (h x) -> p h x", h=HPC)
                bvr = bv_bc[:].rearrange("p (h x) -> p h x", h=HPC)
                nc.vector.tensor_add(vout, psr, bvr)

            def run_interleaved(gens, pattern):
                """Round-robin generators following `pattern` (list of gen
                indices); a finished generator is skipped."""
                live = [iter(g) for g in gens]
                done = [False] * len(gens)
                pi = 0
                while not all(done):
                    g = pattern[pi % len(pattern)]
                    pi += 1
                    if done[g]:
                        continue
                    try:
                        next(live[g])
                    except StopIteration:
                        done[g] = True

            # ---------------- Phase A: m=0 Q/K proj + full V proj ------
            with tc.tile_pool(name="apsum", bufs=2, space="PSUM") as aps:
                for qd in range(4):
                    qk = [emit_qk_group(aps, QT, wqv, bq_sb, 0, qd),
                          emit_qk_group(aps, KT, wkv, bk_sb, 0, qd)]
                    vs = [emit_v_group(aps, s) for s in range(4 * qd, 4 * qd + 4)]
                    # 1 QK mm : 2 V mms keeps V's LDWEIGHTS hidden
                    run_interleaved(qk + vs, [0, 2, 2, 1, 3, 3,
                                              0, 4, 4, 1, 5, 5])

            # ---------------- Phase B: attention, jc descending --------
            with (
                tc.tile_pool(name="scp", bufs=2, space="PSUM") as scp,
                tc.tile_pool(name="opool", bufs=4, space="PSUM") as opool,
            ):
                def qk_m1_filler(dst, wbv, bias):
                    for nb in range(NQB):
                        yield from emit_qk_group(opool, dst, wbv, bias, 1, nb,
                                                 tag="oacc")

                def out0_filler(mo_range):
                    # c=0 half of the output projection (partial sums to
                    # outT0; host adds outT0+outT1) — PE filler during the
                    # ACT-bound attention phase. Output DMAs ride the idle
                    # GpSimd SWDGE ring.
                    for mo in mo_range:
                        ot = stage.tile([P, S], BF16, tag="outstage",
                                        name=f"ot0_{mo}")
                        for qb in range(NQB):
                            ps = opool.tile([P, QB], F32, tag="oacc",
                                            name=f"o0_{mo}_{qb}")
                            yield
                            nc.tensor.matmul(
                                ps[:],
                                wov[:, 0, mo * P:(mo + 1) * P],
                                OT[:, qb * QB:(qb + 1) * QB],
                                start=True, stop=True,
                            )
                            yield
                            nc.vector.tensor_copy(
                                ot[:, qb * QB:(qb + 1) * QB], ps[:])
                            yield
                        nc.gpsimd.dma_start(outT0[mo * P:(mo + 1) * P, :],
                                            ot[:])

                def out1_cols(qb, evict_eng):
                    # one q-column block of the c=1 output projection half
                    for mo in range(D // P):
                        ps = opool.tile([P, QB], F32, tag="oacc",
                                        name=f"o1_{mo}_{qb}")
                        yield
                        nc.tensor.matmul(
                            ps[:],
                            wov[:, 1, mo * P:(mo + 1) * P],
                            OT[:, S + qb * QB: S + (qb + 1) * QB],
                            start=True, stop=True,
                        )
                        yield
                        ot = small.tile([P, QB], BF16, tag="o1s", bufs=4,
                                        name=f"o1s_{mo}_{qb}")
                        evict_eng[mo % len(evict_eng)](ot[:], ps[:])
                        yield
                        nc.gpsimd.dma_start(
                            outT1[mo * P:(mo + 1) * P,
                                  qb * QB:(qb + 1) * QB], ot[:])
                        yield

                def norm_mul(h, m, poff, qb, drecr, o_q):
                    base = 64 if qb >= 2 else 0
                    rbp = opool.tile([64, QB], F32, tag="oacc",
                                     name=f"rbp{h}_{qb}")
                    nc.tensor.matmul(
                        rbp[:],
                        selb[base:base + 33, qb * 64:(qb + 1) * 64],
                        drecr[base:base + 33, :],
                        start=True, stop=True,
                    )
                    nc.vector.tensor_mul(
                        OT[poff:poff + 64,
                           m * S + qb * QB: m * S + (qb + 1) * QB],
                        o_q[:],
                        rbp[:],
                    )

                def head(h, filler=None, pending_norm=None, split_norm=False):
                    m, poff = h // 2, 64 * (h % 2)
                    kt_h = KT[poff:poff + 64, m * S:(m + 1) * S]
                    qt_h = QT[poff:poff + 64, m * S:(m + 1) * S]
                    opsq = {}
                    oqd = {}
                    drec = small.tile([97, QB], F32, tag="drec", bufs=2,
                                      name=f"drec{h}")
                    drecr = small.tile([97, QB], F32, tag="drecr", bufs=2,
                                       name=f"drecr{h}")
                    # unused rows must be finite for the batched reciprocal
                    nc.gpsimd.memset(drec[:], 1.0)
                    for jc in range(SC - 1, -1, -1):
                        if jc == SC - 3 and pending_norm is not None:
                            pending_norm()
                            pending_norm = None
                        # filler only competes for the decay-freed O' psum
                        # slots (jc <= 11), after PV/norm demands are queued
                        if filler is not None and jc <= SC - 5:
                            for _ in range(4):
                                next(filler, None)
                        W = P * (jc + 1)
                        e = epool.tile([P, S], BF16, tag="e")
                        # scores S^T[jc] in 1024-col (2-bank) psum chunks;
                        # diagonal 128-block gets -1e30 masked via an extra
                        # accumulating identity x tri matmul
                        for c0 in range(0, W, EC):
                            cw = min(EC, W - c0)
                            ps = scp.tile([P, EC], F32, tag="sc")
                            for cc in range(0, cw, QB):
                                ccw = min(QB, cw - cc)
                                isdiag = (c0 + cc + ccw == W)
                                nc.tensor.matmul(
                                    ps[:, cc:cc + ccw],
                                    kt_h[:, jc * P:(jc + 1) * P],
                                    qt_h[:, c0 + cc:c0 + cc + ccw],
                                    start=True,
                                    stop=not isdiag,
                                )
                                if isdiag:
                                    d0 = W - P - c0
                                    nc.tensor.matmul(
                                        ps[:, d0:d0 + P],
                                        idb[:],
                                        trib[:],
                                        start=False,
                                        stop=True,
                                        skip_group_check=True,
                                    )
                            nc.scalar.activation(
                                e[:, c0:c0 + cw],
                                ps[:, 0:cw],
                                mybir.ActivationFunctionType.Exp,
                                scale=SCALE,
                            )
                        # PV accumulate; qb block b spans jc = 15 .. 4b
                        for qb in range(jc // 4 + 1):
                            cw = min(QB, W - qb * QB)
                            if jc == SC - 1:
                                opsq[qb] = opool.tile([65, QB], F32, tag="oacc",
                                                      name=f"oacc{h}_{qb}")
                            nc.tensor.matmul(
                                opsq[qb][:, 0:cw],
                                Vb[:, jc * 260 + 65 * h: jc * 260 + 65 * h + 65],
                                e[:, qb * QB:qb * QB + cw],
                                start=(jc == SC - 1),
                                stop=(jc == 4 * qb),
                            )
                            if jc == 4 * qb:
                                # this qb block is finished: stash the
                                # denominator row (to partition 32*qb) and
                                # the unnormalized O' block, free the slot
                                ops = opsq.pop(qb)
                                nc.vector.tensor_copy(
                                    drec[32 * qb:32 * qb + 1, :],
                                    ops[64:65, :])
                                o_q = small.tile([64, QB], BF16, tag="oq",
                                                 bufs=8, name=f"oq{h}_{qb}")
                                nc.vector.tensor_copy(o_q[:], ops[0:64, :])
                                oqd[qb] = o_q
                        if split_norm and jc == 8:
                            # qb3/qb2 denominators are final: normalize
                            # those blocks now and chain their c=1 output
                            # projection columns as additional filler
                            nc.vector.reciprocal(drecr[64:97, :],
                                                 drec[64:97, :])
                            norm_mul(h, m, poff, 3, drecr, oqd[3])
                            norm_mul(h, m, poff, 2, drecr, oqd[2])
                            extra = itertools.chain(
                                out1_cols(3, [nc.vector.tensor_copy,
                                              nc.scalar.copy]),
                                out1_cols(2, [nc.vector.tensor_copy,
                                              nc.scalar.copy]))
                            filler = extra if filler is None else \
                                itertools.chain(filler, extra)
                    if filler is not None:
                        for _ in filler:
                            pass
                    if split_norm:
                        nc.vector.reciprocal(drecr[0:33, :], drec[0:33, :])
                        norm_mul(h, m, poff, 1, drecr, oqd[1])
                        norm_mul(h, m, poff, 0, drecr, oqd[0])
                        for _ in out1_cols(1, [nc.vector.tensor_copy,
                                               nc.scalar.copy]):
                            pass
                        for _ in out1_cols(0, [nc.scalar.copy,
                                               nc.vector.tensor_copy]):
                            pass
                        return None
                    # deferred: one batched reciprocal for the head's 4
                    # denominator rows, then per-block selector-broadcast +
                    # normalize (emitted inside the NEXT head so the PE
                    # stream doesn't stall behind the DVE reciprocal)
                    def norm(h=h, m=m, poff=poff, drec=drec, drecr=drecr,
                             oqd=oqd):
                        nc.vector.reciprocal(drecr[:], drec[:])
                        for qb in range(NQB - 1, -1, -1):
                            norm_mul(h, m, poff, qb, drecr, oqd[qb])
                    return norm

                n0 = head(0, filler=qk_m1_filler(QT, wqv, bq_sb))
                n1 = head(1, filler=qk_m1_filler(KT, wkv, bk_sb),
                          pending_norm=n0)
                n2 = head(2, filler=out0_filler(range(0, 4)), pending_norm=n1)
                head(3, filler=out0_filler(range(4, 8)), pending_norm=n2,
                     split_norm=True)

    _legalize_waits(nc)
    return nc


def _get_nc():
    global _COMPILED
    if _COMPILED is None:
        _COMPILED = _build_nc()
    return _COMPILED


def _make_in_maps(x, wq, bq, wk, bk, wv, bv, wo, bo):
    import ml_dtypes
    bf16 = ml_dtypes.bfloat16
    k = np.arange(P)
    trin = np.where(k[:, None] >= k[None, :], 0.0, -1e30).astype(bf16)
    iden = np.eye(P, dtype=bf16)
    sel = np.zeros((97, 256), dtype=np.float32)
    for r in range(4):
        sel[32 * r, r * 64:(r + 1) * 64] = 1.0
    def pack_w(w):
        # [1024, C] -> [128, (k, C)] : row k*128+p lands at (p, k*C+c)
        C = w.shape[1]
        return np.ascontiguousarray(
            w.astype(bf16).reshape(KC, P, C).transpose(1, 0, 2).reshape(P, KC * C))

    xTs = []
    for b in range(B):
        # x^T [1024, 2048] -> [128, (qd, k, s')]
        xt = np.ascontiguousarray(x[b].T).astype(bf16)
        xTs.append(np.ascontiguousarray(
            xt.reshape(KC, P, 4, 512).transpose(1, 2, 0, 3).reshape(P, KC * S)))
    in_maps = []
    for c in range(NCORES):
        b, g = c // 4, c % 4
        cols = slice(DHC * g, DHC * (g + 1))
        wo_g = wo[cols, :]  # [256, 1024]
        wo_p = np.ascontiguousarray(
            wo_g.astype(bf16).reshape(2, P, D).transpose(1, 0, 2).reshape(P, 2 * D))
        in_maps.append({
            "xT": xTs[b],
            "wq": pack_w(np.asarray(wq[:, cols])),
            "wk": pack_w(np.asarray(wk[:, cols])),
            "wv": pack_w(np.asarray(wv[:, cols])),
            "wo": wo_p,
            "bq": np.ascontiguousarray(bq[cols]).reshape(2, P, 1),
            "bk": np.ascontiguousarray(bk[cols]).reshape(2, P, 1),
            "bv": np.ascontiguousarray(np.broadcast_to(bv[cols].reshape(1, DHC), (P, DHC))),
            "trin": trin,
            "iden": iden,
            "sel": sel,
        })
    return in_maps


def kernel(x, wq, bq, wk, bk, wv, bv, wo, bo, _trace=False, _trace_kwargs=None):
    x = np.asarray(x, dtype=np.float32)
    assert x.shape == (B, S, D), x.shape
    nc = _get_nc()
    in_maps = _make_in_maps(
        x, np.asarray(wq), np.asarray(bq), np.asarray(wk), np.asarray(bk),
        np.asarray(wv), np.asarray(bv), np.asarray(wo), np.asarray(bo))
    kw = {}
    if _trace:
        kw = dict(trace=True, **(_trace_kwargs or {}))
    res = run_bass_kernel_spmd(nc, in_maps, list(range(NCORES)), **kw)
    out = np.empty((B, S, D), dtype=np.float32)
    for b in range(B):
        acc = np.zeros((D, S), dtype=np.float64)
        for g in range(4):
            acc += res.results[4 * b + g]["outT0"].astype(np.float64)
            acc += res.results[4 * b + g]["outT1"].astype(np.float64)
        out[b] = acc.T.astype(np.float32) + np.asarray(bo, dtype=np.float32)
    kernel.last_result = res
    return out


# revision 45
# speedup vs baseline: 1.0243x; 1.0243x over previous
"""Trainium2 Bass kernel for 16-head self-attention (D=1024, S=2048, B=2)
with upper-triangular (j >= i) mask and scale 1/head_dim.

Sharding: batch*head-group parallel over 8 cores. Core c handles batch
c//4, heads [4*(c%4), 4*(c%4)+4). Each core computes Q/K/V projections for
its 256 output dims, attention for its 4 heads, and a partial output
projection (its 256 rows of wo). Host sums the 4 partials per batch.

On-chip layout is transposed end-to-end: QT/KT [dh, seq], scores S^T
[seq_k, seq_q] (stationary=K^T chunk, moving=Q^T), exp on ScalarE
PSUM->SBUF with the 1/64 scale folded in, PV as O'^T = V'^T E^T with a
ones-column appended to V so row 64 of O' is the softmax denominator,
then out^T = wo^T O^T (bf16 partials). The host transposes back.

Perf structure (v2):
- mask applied inside the score PSUM accumulation via an extra
  identity x (-1e30 upper-tri) matmul -> no post-exp mask stage.
- attention iterates key chunks jc DESCENDING so the per-qb O' PSUM
  banks free progressively; each qb block normalizes early with
  reciprocal_approx_fast + a K=1 ones matmul broadcast.
- exp runs on [128, 1024] two-bank PSUM chunks (fewer ACTIVATEs).
- Q/K (m=0) and V projections interleave 1:2 so V's per-matmul
  LDWEIGHTS hides under the 512-wide Q/K matmuls; m=1 projections are
  emitted between attention heads as PE filler (and reuse freed O'
  PSUM slots), keeping the PE dense enough to hold the HAM clock warm.
- batched input DMAs (one per weight tensor, x in 4 column quarters).
"""

import itertools
import sys

sys.path.insert(0, "/opt/trn_rl_repo")

import numpy as np

import concourse.bass as bass
import concourse.mybir as mybir
from concourse import tile
from concourse.bass_utils import run_bass_kernel_spmd

# ---------------------------------------------------------------------------
# Workaround: this walrus build supports only 1 sync wait on the SP CTRL
# (drain) instruction; split the TileContext exit drain's waits across
# sequential drains (same-engine program order makes this equivalent).
_MAX_DRAIN_WAITS = 1


def _patched_drain_and_barrier(self, tick_clock, wait_clock):
    from bass_rust import ScopedClock

    nc = self.nc
    drain_inst = nc.sync.drain()
    wait_clock.add_sem_waits(
        drain_inst.ins, ScopedClock({None: tick_clock.global_clock})
    )
    si = drain_inst.ins.sync_info
    if si is not None and len(si.on_wait) > _MAX_DRAIN_WAITS:
        waits = list(si.on_wait)
        si.on_wait = waits[:_MAX_DRAIN_WAITS]
        rest = waits[_MAX_DRAIN_WAITS:]
        while rest:
            chunk, rest = rest[:_MAX_DRAIN_WAITS], rest[_MAX_DRAIN_WAITS:]
            extra = nc.sync.drain()
            esi = extra.ins.sync_info
            if esi is None:
                extra.ins.sync_info = mybir.SyncInfo(on_wait=chunk, on_update=[])
            else:
                esi.on_wait = chunk
    nc.all_engine_barrier()
    assert self.sems is not None
    popped = nc._tile_sem_poison_stack.pop()
    assert popped is self._sem_poison
    nc.clear_and_free_semaphores(list(self.sems.allocated().values()))
    nc.all_engine_barrier()


tile.TileContext._drain_and_barrier = _patched_drain_and_barrier


def _legalize_waits(nc, max_waits=1):
    """This walrus build accepts at most one sync wait per instruction.
    Hoist extra waits onto preceding NoOps on the same engine (same-engine
    program order preserves the gating semantics)."""
    for blk in nc.main_func.blocks:
        out = []
        for inst in blk.instructions:
            si = inst.sync_info
            if si is not None and len(si.on_wait) > max_waits:
                waits = list(si.on_wait)
                si.on_wait = waits[-max_waits:]
                for w in waits[:-max_waits]:
                    nop = mybir.InstNoOp(
                        name=nc.get_next_instruction_name(), ins=[], outs=[]
                    )
                    nop.engine = inst.engine
                    nop.sync_info = mybir.SyncInfo(on_wait=[w], on_update=[])
                    nc.register_instruction(nop)
                    out.append(nop)
            out.append(inst)
        blk.instructions[:] = out


# ---------------------------------------------------------------------------

B, S, D = 2, 2048, 1024
H, HD = 16, 64
SCALE = 1.0 / HD
NCORES = 8
HPC = 4          # heads per core
DHC = HPC * HD   # 256 head-dims per core
P = 128
KC = D // P      # 8 contraction chunks for projections
SC = S // P      # 16 seq chunks of 128
QB = 512         # seq_q block for PV / O-proj
NQB = S // QB    # 4
EC = 1024        # exp chunk width (2 PSUM banks)

F32 = mybir.dt.float32
F32R = mybir.dt.float32r
BF16 = mybir.dt.bfloat16

_COMPILED = None


def _build_nc():
    nc = bass.Bass("TRN2", target_bir_lowering=False, debug=False,
                   num_devices=NCORES)

    # All big inputs are pre-packed host-side into the exact [128, N]
    # SBUF layouts so every load is a contiguous max-rate 2D DMA.
    # xT packed as [p, (quarter, k, s')]: col = qd*4096 + k*512 + s'
    xT = nc.declare_dram_parameter("xT", [P, KC * S], BF16, isOutput=False)
    wq = nc.declare_dram_parameter("wq", [P, KC * DHC], BF16, isOutput=False)
    wk = nc.declare_dram_parameter("wk", [P, KC * DHC], BF16, isOutput=False)
    wv = nc.declare_dram_parameter("wv", [P, KC * DHC], BF16, isOutput=False)
    wo = nc.declare_dram_parameter("wo", [P, 2 * D], BF16, isOutput=False)
    bq = nc.declare_dram_parameter("bq", [2, P, 1], F32, isOutput=False)
    bk = nc.declare_dram_parameter("bk", [2, P, 1], F32, isOutput=False)
    bv = nc.declare_dram_parameter("bv", [P, DHC], F32, isOutput=False)
    trin = nc.declare_dram_parameter("trin", [P, P], BF16, isOutput=False)
    iden = nc.declare_dram_parameter("iden", [P, P], BF16, isOutput=False)
    sel = nc.declare_dram_parameter("sel", [97, 4 * 64], F32, isOutput=False)
    outT0 = nc.declare_dram_parameter("outT0", [D, S], BF16, isOutput=True)
    outT1 = nc.declare_dram_parameter("outT1", [D, S], BF16, isOutput=True)

    with tile.TileContext(nc) as tc:
        dmaq = [nc.sync, nc.scalar]
        dq = [0]

        def dma(out_ap, in_ap):
            eng = dmaq[dq[0] % len(dmaq)]
            dq[0] += 1
            return eng.dma_start(out_ap, in_ap)

        with (
            tc.tile_pool(name="persist", bufs=1) as pp,
            tc.tile_pool(name="stage", bufs=2) as stage,
            tc.tile_pool(name="epool", bufs=4) as epool,
            tc.tile_pool(name="small", bufs=4) as small,
        ):
            # ---------------- persistent SBUF tensors ----------------
            xb = pp.tile([P, KC * S], BF16, tag="xb")          # 32KB
            wqb = pp.tile([P, KC * DHC], BF16, tag="wqb")      # 4KB
            wkb = pp.tile([P, KC * DHC], BF16, tag="wkb")
            wvb = pp.tile([P, KC * DHC], BF16, tag="wvb")
            wob = pp.tile([P, 2 * D], BF16, tag="wob")
            QT = pp.tile([P, 2 * S], BF16, tag="qt")           # 8KB
            KT = pp.tile([P, 2 * S], BF16, tag="kt")
            # V with a ones column per head: 16 chunks x [h0(64) 1 | ...]
            Vb = pp.tile([P, SC * HPC * 65], BF16, tag="vb")   # 8.3KB
            OT = pp.tile([P, 2 * S], BF16, tag="ot")
            trib = pp.tile([P, P], BF16, tag="trib")
            idb = pp.tile([P, P], BF16, tag="idb")
            bq_sb = pp.tile([P, 2], F32, tag="bq")
            bk_sb = pp.tile([P, 2], F32, tag="bk")
            bv_bc = pp.tile([P, DHC], F32, tag="bvbc")
            # One-hot selector: sel[:, 64r:64r+64] has row 32r all-ones
            # (denominator rows sit at 32-aligned partitions; the K=97
            # matmul broadcasts row 32r of drecr to 64 partitions).
            selb = pp.tile([97, 4 * 64], F32, tag="selb")

            # xb layout [p, (qd, k, s')]: QK group nb reads
            # xq[:, nb, k, :]; V s-chunk reads xq[:, s//4, k, (s%4)*128..]
            xq = xb[:].rearrange("p (q k s) -> p q k s", q=4, k=KC)
            wqv = wqb[:].rearrange("p (k c) -> p k c", k=KC)
            wkv = wkb[:].rearrange("p (k c) -> p k c", k=KC)
            wvv = wvb[:].rearrange("p (k c) -> p k c", k=KC)
            wov = wob[:].rearrange("p (c d) -> p c d", c=2)

            # ---------------- input DMAs (all contiguous 2D) ----------
            nc.sync.dma_start(bq_sb[:, 0:1], bq[0])
            nc.sync.dma_start(bq_sb[:, 1:2], bq[1])
            nc.scalar.dma_start(bk_sb[:, 0:1], bk[0])
            nc.scalar.dma_start(bk_sb[:, 1:2], bk[1])
            # sync ring: x quarters (the critical path); scalar: weights
            for qd in range(4):
                nc.sync.dma_start(xb[:, qd * 4096:(qd + 1) * 4096],
                                  xT[:, qd * 4096:(qd + 1) * 4096])
            nc.scalar.dma_start(wqb[:], wq[:, :])
            nc.scalar.dma_start(wvb[:], wv[:, :])
            nc.scalar.dma_start(wkb[:], wk[:, :])
            nc.scalar.dma_start(bv_bc[:], bv[:, :])
            nc.scalar.dma_start(trib[:], trin[:, :])
            nc.scalar.dma_start(idb[:], iden[:, :])
            nc.scalar.dma_start(wob[:], wo[:, :])
            nc.scalar.dma_start(selb[:], sel[:, :])
            # ones column of V (col 64 of each head's 65-block)
            nc.gpsimd.memset(
                Vb[:].rearrange("p (s h x) -> p s h x", s=SC, h=HPC)
                [:, :, :, 64:65], 1.0)

            # ---------------- helpers ----------------
            def emit_qk_group(aps, dst, wbv, bias, m, nb, tag="pqk"):
                ps = aps.tile([P, QB], F32, tag=tag, name=f"pqk{m}_{nb}")
                yield  # allow interleave before the first MM
                for k in range(KC):
                    nc.tensor.matmul(
                        ps[:],
                        wbv[:, k, m * P:(m + 1) * P],
                        xq[:, nb, k, :],
                        start=(k == 0),
                        stop=(k == KC - 1),
                    )
                    yield
                nc.vector.tensor_scalar_add(
                    dst[:, m * S + nb * QB: m * S + (nb + 1) * QB],
                    ps[:],
                    bias[:, m:m + 1],
                )

            def emit_v_group(aps, s):
                ps = aps.tile([P, DHC], F32, tag="pv")
                yield
                for k in range(KC):
                    nc.tensor.matmul(
                        ps[:],
                        xq[:, s // 4, k, (s % 4) * P:(s % 4 + 1) * P],
                        wvv[:, k, :],
                        start=(k == 0),
                        stop=(k == KC - 1),
                    )
                    yield
                vout = Vb[:, s * 260:(s + 1) * 260].rearrange(
                    "p (h x) -> p h x", h=HPC)[:, :, 0:64]
                psr = ps[:].rearrange("p (h x) -> p h x", h=HPC)
                bvr = bv_bc[:].rearrange("p (h x) -> p h x", h=HPC)
                nc.vector.tensor_add(vout, psr, bvr)

            def run_interleaved(gens, pattern):
                """Round-robin generators following `pattern` (list of gen
                indices); a finished generator is skipped."""
                live = [iter(g) for g in gens]
                done = [False] * len(gens)
                pi = 0
                while not all(done):
                    g = pattern[pi % len(pattern)]
                    pi += 1
                    if done[g]:
                        continue
                    try:
                        next(live[g])
                    except StopIteration:
                        done[g] = True

            # ---------------- Phase A: m=0 Q/K proj + full V proj ------
            with tc.tile_pool(name="apsum", bufs=2, space="PSUM") as aps:
                for qd in range(4):
                    qk = [emit_qk_group(aps, QT, wqv, bq_sb, 0, qd),
                          emit_qk_group(aps, KT, wkv, bk_sb, 0, qd)]
                    vs = [emit_v_group(aps, s) for s in range(4 * qd, 4 * qd + 4)]
                    # 1 QK mm : 2 V mms keeps V's LDWEIGHTS hidden
                    run_interleaved(qk + vs, [0, 2, 2, 1, 3, 3,
                                              0, 4, 4, 1, 5, 5])

            # ---------------- Phase B: attention, jc descending --------
            with (
                tc.tile_pool(name="scp", bufs=2, space="PSUM") as scp,
                tc.tile_pool(name="opool", bufs=4, space="PSUM") as opool,
            ):
                def qk_m1_filler(dst, wbv, bias):
                    for nb in range(NQB):
                        yield from emit_qk_group(opool, dst, wbv, bias, 1, nb,
                                                 tag="oacc")

                def out0_filler(mo_range):
                    # c=0 half of the output projection (partial sums to
                    # outT0; host adds outT0+outT1) — PE filler during the
                    # ACT-bound attention phase. Output DMAs ride the idle
                    # GpSimd SWDGE ring.
                    for mo in mo_range:
                        ot = stage.tile([P, S], BF16, tag="outstage",
                                        name=f"ot0_{mo}")
                        for qb in range(NQB):
                            ps = opool.tile([P, QB], F32, tag="oacc",
                                            name=f"o0_{mo}_{qb}")
                            yield
                            nc.tensor.matmul(
                                ps[:],
                                wov[:, 0, mo * P:(mo + 1) * P],
                                OT[:, qb * QB:(qb + 1) * QB],
                                start=True, stop=True,
                            )
                            yield
                            nc.vector.tensor_copy(
                                ot[:, qb * QB:(qb + 1) * QB], ps[:])
                            yield
                        nc.gpsimd.dma_start(outT0[mo * P:(mo + 1) * P, :],
                                            ot[:])

                def norm_mul(h, m, poff, qb, drecr, o_q):
                    base = 64 if qb >= 2 else 0
                    rbp = opool.tile([64, QB], F32, tag="oacc",
                                     name=f"rbp{h}_{qb}")
                    nc.tensor.matmul(
                        rbp[:],
                        selb[base:base + 33, qb * 64:(qb + 1) * 64],
                        drecr[base:base + 33, :],
                        start=True, stop=True,
                    )
                    nc.vector.tensor_mul(
                        OT[poff:poff + 64,
                           m * S + qb * QB: m * S + (qb + 1) * QB],
                        o_q[:],
                        rbp[:],
                    )

                def head(h, filler=None, pending_norm=None, split_norm=False):
                    m, poff = h // 2, 64 * (h % 2)
                    kt_h = KT[poff:poff + 64, m * S:(m + 1) * S]
                    qt_h = QT[poff:poff + 64, m * S:(m + 1) * S]
                    opsq = {}
                    oqd = {}
                    drec = small.tile([97, QB], F32, tag="drec", bufs=2,
                                      name=f"drec{h}")
                    drecr = small.tile([97, QB], F32, tag="drecr", bufs=2,
                                       name=f"drecr{h}")
                    # unused rows must be finite for the batched reciprocal
                    nc.gpsimd.memset(drec[:], 1.0)
                    for jc in range(SC - 1, -1, -1):
                        if jc == SC - 3 and pending_norm is not None:
                            pending_norm()
                            pending_norm = None
                        # filler only competes for the decay-freed O' psum
                        # slots (jc <= 11), after PV/norm demands are queued
                        if filler is not None and jc <= SC - 5:
                            for _ in range(6):
                                next(filler, None)
                        W = P * (jc + 1)
                        e = epool.tile([P, S], BF16, tag="e")
                        # scores S^T[jc] in 1024-col (2-bank) psum chunks;
                        # diagonal 128-block gets -1e30 masked via an extra
                        # accumulating identity x tri matmul
                        for c0 in range(0, W, EC):
                            cw = min(EC, W - c0)
                            ps = scp.tile([P, EC], F32, tag="sc")
                            for cc in range(0, cw, QB):
                                ccw = min(QB, cw - cc)
                                isdiag = (c0 + cc + ccw == W)
                                nc.tensor.matmul(
                                    ps[:, cc:cc + ccw],
                                    kt_h[:, jc * P:(jc + 1) * P],
                                    qt_h[:, c0 + cc:c0 + cc + ccw],
                                    start=True,
                                    stop=not isdiag,
                                )
                                if isdiag:
                                    d0 = W - P - c0
                                    nc.tensor.matmul(
                                        ps[:, d0:d0 + P],
                                        idb[:],
                                        trib[:],
                                        start=False,
                                        stop=True,
                                        skip_group_check=True,
                                    )
                            nc.scalar.activation(
                                e[:, c0:c0 + cw],
                                ps[:, 0:cw],
                                mybir.ActivationFunctionType.Exp,
                                scale=SCALE,
                            )
                        # PV accumulate; qb block b spans jc = 15 .. 4b
                        for qb in range(jc // 4 + 1):
                            cw = min(QB, W - qb * QB)
                            if jc == SC - 1:
                                opsq[qb] = opool.tile([65, QB], F32, tag="oacc",
                                                      name=f"oacc{h}_{qb}")
                            nc.tensor.matmul(
                                opsq[qb][:, 0:cw],
                                Vb[:, jc * 260 + 65 * h: jc * 260 + 65 * h + 65],
                                e[:, qb * QB:qb * QB + cw],
                                start=(jc == SC - 1),
                                stop=(jc == 4 * qb),
                            )
                            if jc == 4 * qb:
                                # this qb block is finished: stash the
                                # denominator row (to partition 32*qb) and
                                # the unnormalized O' block, free the slot
                                ops = opsq.pop(qb)
                                nc.vector.tensor_copy(
                                    drec[32 * qb:32 * qb + 1, :],
                                    ops[64:65, :])
                                o_q = small.tile([64, QB], BF16, tag="oq",
                                                 bufs=8, name=f"oq{h}_{qb}")
                                nc.vector.tensor_copy(o_q[:], ops[0:64, :])
                                oqd[qb] = o_q
                        if split_norm and jc == 8:
                            # qb3/qb2 denominators are final: normalize
                            # those blocks now so the tail's c=1 output
                            # projection can start with them immediately
                            nc.vector.reciprocal(drecr[64:97, :],
                                                 drec[64:97, :])
                            norm_mul(h, m, poff, 3, drecr, oqd[3])
                            norm_mul(h, m, poff, 2, drecr, oqd[2])
                    if filler is not None:
                        for _ in filler:
                            pass
                    if split_norm:
                        nc.vector.reciprocal(drecr[0:33, :], drec[0:33, :])
                        norm_mul(h, m, poff, 1, drecr, oqd[1])
                        norm_mul(h, m, poff, 0, drecr, oqd[0])
                        return None
                    # deferred: one batched reciprocal for the head's 4
                    # denominator rows, then per-block selector-broadcast +
                    # normalize (emitted inside the NEXT head so the PE
                    # stream doesn't stall behind the DVE reciprocal)
                    def norm(h=h, m=m, poff=poff, drec=drec, drecr=drecr,
                             oqd=oqd):
                        nc.vector.reciprocal(drecr[:], drec[:])
                        for qb in range(NQB - 1, -1, -1):
                            norm_mul(h, m, poff, qb, drecr, oqd[qb])
                    return norm

                n0 = head(0, filler=qk_m1_filler(QT, wqv, bq_sb))
                n1 = head(1, filler=qk_m1_filler(KT, wkv, bk_sb),
                          pending_norm=n0)
                n2 = head(2, filler=out0_filler(range(0, 4)), pending_norm=n1)
                head(3, filler=out0_filler(range(4, 8)), pending_norm=n2,
                     split_norm=True)

            # ---------------- Phase C: c=1 output projection half -------
            # qb-outer, descending: qb3/qb2 were normalized mid-head-3, so
            # their matmuls start without waiting for the final reciprocal
            with tc.tile_pool(name="cpsum", bufs=4, space="PSUM") as cps:
                for qb in range(NQB - 1, -1, -1):
                    for mo in range(D // P):
                        ps = cps.tile([P, QB], F32, tag="oproj")
                        nc.tensor.matmul(
                            ps[:],
                            wov[:, 1, mo * P:(mo + 1) * P],
                            OT[:, S + qb * QB: S + (qb + 1) * QB],
                            start=True, stop=True,
                        )
                        ot = small.tile([P, QB], BF16, tag="o1s", bufs=8,
                                        name=f"o1s_{mo}_{qb}")
                        if mo % 2 == 0:
                            nc.scalar.copy(ot[:], ps[:])
                        else:
                            nc.vector.tensor_copy(ot[:], ps[:])
                        nc.gpsimd.dma_start(
                            outT1[mo * P:(mo + 1) * P,
                                  qb * QB:(qb + 1) * QB], ot[:])

    _legalize_waits(nc)
    return nc


def _get_nc():
    global _COMPILED
    if _COMPILED is None:
        _COMPILED = _build_nc()
    return _COMPILED


def _make_in_maps(x, wq, bq, wk, bk, wv, bv, wo, bo):
    import ml_dtypes
    bf16 = ml_dtypes.bfloat16
    k = np.arange(P)
    trin = np.where(k[:, None] >= k[None, :], 0.0, -1e30).astype(bf16)
    iden = np.eye(P, dtype=bf16)
    sel = np.zeros((97, 256), dtype=np.float32)
    for r in range(4):
        sel[32 * r, r * 64:(r + 1) * 64] = 1.0
    def pack_w(w):
        # [1024, C] -> [128, (k, C)] : row k*128+p lands at (p, k*C+c)
        C = w.shape[1]
        return np.ascontiguousarray(
            w.astype(bf16).reshape(KC, P, C).transpose(1, 0, 2).reshape(P, KC * C))

    xTs = []
    for b in range(B):
        # x^T [1024, 2048] -> [128, (qd, k, s')]
        xt = np.ascontiguousarray(x[b].T).astype(bf16)
        xTs.append(np.ascontiguousarray(
            xt.reshape(KC, P, 4, 512).transpose(1, 2, 0, 3).reshape(P, KC * S)))
    in_maps = []
    for c in range(NCORES):
        b, g = c // 4, c % 4
        cols = slice(DHC * g, DHC * (g + 1))
        wo_g = wo[cols, :]  # [256, 1024]
        wo_p = np.ascontiguousarray(
            wo_g.astype(bf16).reshape(2, P, D).transpose(1, 0, 2).reshape(P, 2 * D))
        in_maps.append({
            "xT": xTs[b],
            "wq": pack_w(np.asarray(wq[:, cols])),
            "wk": pack_w(np.asarray(wk[:, cols])),
            "wv": pack_w(np.asarray(wv[:, cols])),
            "wo": wo_p,
            "bq": np.ascontiguousarray(bq[cols]).reshape(2, P, 1),
            "bk": np.ascontiguousarray(bk[cols]).reshape(2, P, 1),
            "bv": np.ascontiguousarray(np.broadcast_to(bv[cols].reshape(1, DHC), (P, DHC))),
            "trin": trin,
            "iden": iden,
            "sel": sel,
        })
    return in_maps


def kernel(x, wq, bq, wk, bk, wv, bv, wo, bo, _trace=False, _trace_kwargs=None):
    x = np.asarray(x, dtype=np.float32)
    assert x.shape == (B, S, D), x.shape
    nc = _get_nc()
    in_maps = _make_in_maps(
        x, np.asarray(wq), np.asarray(bq), np.asarray(wk), np.asarray(bk),
        np.asarray(wv), np.asarray(bv), np.asarray(wo), np.asarray(bo))
    kw = {}
    if _trace:
        kw = dict(trace=True, **(_trace_kwargs or {}))
    res = run_bass_kernel_spmd(nc, in_maps, list(range(NCORES)), **kw)
    out = np.empty((B, S, D), dtype=np.float32)
    for b in range(B):
        acc = np.zeros((D, S), dtype=np.float64)
        for g in range(4):
            acc += res.results[4 * b + g]["outT0"].astype(np.float64)
            acc += res.results[4 * b + g]["outT1"].astype(np.float64)
        out[b] = acc.T.astype(np.float32) + np.asarray(bo, dtype=np.float32)
    kernel.last_result = res
    return out


# revision 55
# speedup vs baseline: 1.1415x; 1.1145x over previous
"""Trainium2 Bass kernel for 16-head self-attention (D=1024, S=2048, B=2)
with upper-triangular (j >= i) mask and scale 1/head_dim.

Sharding: batch*head-group parallel over 8 cores. Core c handles batch
c//4, heads [4*(c%4), 4*(c%4)+4). Each core computes Q/K/V projections for
its 256 output dims, attention for its 4 heads, and a partial output
projection (its 256 rows of wo). Host sums the 4 partials per batch.

On-chip layout is transposed end-to-end: QT/KT [dh, seq], scores S^T
[seq_k, seq_q] (stationary=K^T chunk, moving=Q^T), exp on ScalarE
PSUM->SBUF with the 1/64 scale folded in, PV as O'^T = V'^T E^T with a
ones-column appended to V so row 64 of O' is the softmax denominator,
then out^T = wo^T O^T (bf16 partials). The host transposes back.

Perf structure (v2):
- mask applied inside the score PSUM accumulation via an extra
  identity x (-1e30 upper-tri) matmul -> no post-exp mask stage.
- attention iterates key chunks jc DESCENDING so the per-qb O' PSUM
  banks free progressively; each qb block normalizes early with
  reciprocal_approx_fast + a K=1 ones matmul broadcast.
- exp runs on [128, 1024] two-bank PSUM chunks (fewer ACTIVATEs).
- Q/K (m=0) and V projections interleave 1:2 so V's per-matmul
  LDWEIGHTS hides under the 512-wide Q/K matmuls; m=1 projections are
  emitted between attention heads as PE filler (and reuse freed O'
  PSUM slots), keeping the PE dense enough to hold the HAM clock warm.
- batched input DMAs (one per weight tensor, x in 4 column quarters).
"""

import itertools
import sys

sys.path.insert(0, "/opt/trn_rl_repo")

import numpy as np

import concourse.bass as bass
import concourse.mybir as mybir
from concourse import tile
from concourse.bass_utils import run_bass_kernel_spmd

# ---------------------------------------------------------------------------
# Workaround: this walrus build supports only 1 sync wait on the SP CTRL
# (drain) instruction; split the TileContext exit drain's waits across
# sequential drains (same-engine program order makes this equivalent).
_MAX_DRAIN_WAITS = 1


def _patched_drain_and_barrier(self, tick_clock, wait_clock):
    from bass_rust import ScopedClock

    nc = self.nc
    drain_inst = nc.sync.drain()
    wait_clock.add_sem_waits(
        drain_inst.ins, ScopedClock({None: tick_clock.global_clock})
    )
    si = drain_inst.ins.sync_info
    if si is not None and len(si.on_wait) > _MAX_DRAIN_WAITS:
        waits = list(si.on_wait)
        si.on_wait = waits[:_MAX_DRAIN_WAITS]
        rest = waits[_MAX_DRAIN_WAITS:]
        while rest:
            chunk, rest = rest[:_MAX_DRAIN_WAITS], rest[_MAX_DRAIN_WAITS:]
            extra = nc.sync.drain()
            esi = extra.ins.sync_info
            if esi is None:
                extra.ins.sync_info = mybir.SyncInfo(on_wait=chunk, on_update=[])
            else:
                esi.on_wait = chunk
    nc.all_engine_barrier()
    assert self.sems is not None
    popped = nc._tile_sem_poison_stack.pop()
    assert popped is self._sem_poison
    nc.clear_and_free_semaphores(list(self.sems.allocated().values()))
    nc.all_engine_barrier()


tile.TileContext._drain_and_barrier = _patched_drain_and_barrier


def _legalize_waits(nc, max_waits=1):
    """This walrus build accepts at most one sync wait per instruction.
    Hoist extra waits onto preceding NoOps on the same engine (same-engine
    program order preserves the gating semantics)."""
    for blk in nc.main_func.blocks:
        out = []
        for inst in blk.instructions:
            si = inst.sync_info
            if si is not None and len(si.on_wait) > max_waits:
                waits = list(si.on_wait)
                si.on_wait = waits[-max_waits:]
                for w in waits[:-max_waits]:
                    nop = mybir.InstNoOp(
                        name=nc.get_next_instruction_name(), ins=[], outs=[]
                    )
                    nop.engine = inst.engine
                    nop.sync_info = mybir.SyncInfo(on_wait=[w], on_update=[])
                    nc.register_instruction(nop)
                    out.append(nop)
            out.append(inst)
        blk.instructions[:] = out


# ---------------------------------------------------------------------------

B, S, D = 2, 2048, 1024
H, HD = 16, 64
SCALE = 1.0 / HD
NCORES = 8
HPC = 4          # heads per core
DHC = HPC * HD   # 256 head-dims per core
P = 128
KC = D // P      # 8 contraction chunks for projections
SC = S // P      # 16 seq chunks of 128
QB = 512         # seq_q block for PV / O-proj
NQB = S // QB    # 4
EC = 1024        # exp chunk width (2 PSUM banks)

F32 = mybir.dt.float32
F32R = mybir.dt.float32r
BF16 = mybir.dt.bfloat16

_COMPILED = None


def _build_nc():
    nc = bass.Bass("TRN2", target_bir_lowering=False, debug=False,
                   num_devices=NCORES)

    # All big inputs are pre-packed host-side into the exact [128, N]
    # SBUF layouts so every load is a contiguous max-rate 2D DMA.
    # xT packed as [p, (quarter, k, s')]: col = qd*4096 + k*512 + s'
    xT = nc.declare_dram_parameter("xT", [P, KC * S], BF16, isOutput=False)
    wq = nc.declare_dram_parameter("wq", [P, KC * DHC], BF16, isOutput=False)
    wk = nc.declare_dram_parameter("wk", [P, KC * DHC], BF16, isOutput=False)
    wv = nc.declare_dram_parameter("wv", [P, KC * DHC], BF16, isOutput=False)
    wo = nc.declare_dram_parameter("wo", [P, 2 * D], BF16, isOutput=False)
    bq = nc.declare_dram_parameter("bq", [2, P, 1], F32, isOutput=False)
    bk = nc.declare_dram_parameter("bk", [2, P, 1], F32, isOutput=False)
    bv = nc.declare_dram_parameter("bv", [P, DHC], F32, isOutput=False)
    trin = nc.declare_dram_parameter("trin", [P, P], BF16, isOutput=False)
    iden = nc.declare_dram_parameter("iden", [P, P], BF16, isOutput=False)
    sel = nc.declare_dram_parameter("sel", [97, 4 * 64], BF16, isOutput=False)
    outT0 = nc.declare_dram_parameter("outT0", [D, S], BF16, isOutput=True)
    outT1 = nc.declare_dram_parameter("outT1", [D, S], BF16, isOutput=True)

    with tile.TileContext(nc) as tc:
        dmaq = [nc.sync, nc.scalar]
        dq = [0]

        def dma(out_ap, in_ap):
            eng = dmaq[dq[0] % len(dmaq)]
            dq[0] += 1
            return eng.dma_start(out_ap, in_ap)

        with (
            tc.tile_pool(name="persist", bufs=1) as pp,
            tc.tile_pool(name="stage", bufs=2) as stage,
            tc.tile_pool(name="epool", bufs=4) as epool,
            tc.tile_pool(name="small", bufs=4) as small,
        ):
            # ---------------- persistent SBUF tensors ----------------
            xb = pp.tile([P, KC * S], BF16, tag="xb")          # 32KB
            wqb = pp.tile([P, KC * DHC], BF16, tag="wqb")      # 4KB
            wkb = pp.tile([P, KC * DHC], BF16, tag="wkb")
            wvb = pp.tile([P, KC * DHC], BF16, tag="wvb")
            wob = pp.tile([P, 2 * D], BF16, tag="wob")
            QT = pp.tile([P, 2 * S], BF16, tag="qt")           # 8KB
            KT = pp.tile([P, 2 * S], BF16, tag="kt")
            # V with a ones column per head: 16 chunks x [h0(64) 1 | ...]
            Vb = pp.tile([P, SC * HPC * 65], BF16, tag="vb")   # 8.3KB
            OT = pp.tile([P, 2 * S], BF16, tag="ot")
            trib = pp.tile([P, P], BF16, tag="trib")
            idb = pp.tile([P, P], BF16, tag="idb")
            bq_sb = pp.tile([P, 2], F32, tag="bq")
            bk_sb = pp.tile([P, 2], F32, tag="bk")
            bv_bc = pp.tile([P, DHC], F32, tag="bvbc")
            # One-hot selector: sel[:, 64r:64r+64] has row 32r all-ones
            # (denominator rows sit at 32-aligned partitions; the K=97
            # matmul broadcasts row 32r of drecr to 64 partitions).
            selb = pp.tile([97, 4 * 64], BF16, tag="selb")

            # xb layout [p, (qd, k, s')]: QK group nb reads
            # xq[:, nb, k, :]; V s-chunk reads xq[:, s//4, k, (s%4)*128..]
            xq = xb[:].rearrange("p (q k s) -> p q k s", q=4, k=KC)
            wqv = wqb[:].rearrange("p (k c) -> p k c", k=KC)
            wkv = wkb[:].rearrange("p (k c) -> p k c", k=KC)
            wvv = wvb[:].rearrange("p (k c) -> p k c", k=KC)
            wov = wob[:].rearrange("p (c d) -> p c d", c=2)

            # ---------------- input DMAs (all contiguous 2D) ----------
            nc.sync.dma_start(bq_sb[:, 0:1], bq[0])
            nc.sync.dma_start(bq_sb[:, 1:2], bq[1])
            nc.scalar.dma_start(bk_sb[:, 0:1], bk[0])
            nc.scalar.dma_start(bk_sb[:, 1:2], bk[1])
            # sync ring: x quarters (the critical path); scalar: weights
            for qd in range(4):
                nc.sync.dma_start(xb[:, qd * 4096:(qd + 1) * 4096],
                                  xT[:, qd * 4096:(qd + 1) * 4096])
            nc.scalar.dma_start(wqb[:], wq[:, :])
            nc.scalar.dma_start(wvb[:], wv[:, :])
            nc.scalar.dma_start(wkb[:], wk[:, :])
            nc.scalar.dma_start(bv_bc[:], bv[:, :])
            nc.scalar.dma_start(trib[:], trin[:, :])
            nc.scalar.dma_start(idb[:], iden[:, :])
            nc.scalar.dma_start(wob[:], wo[:, :])
            nc.scalar.dma_start(selb[:], sel[:, :])
            # ones column of V (col 64 of each head's 65-block)
            nc.gpsimd.memset(
                Vb[:].rearrange("p (s h x) -> p s h x", s=SC, h=HPC)
                [:, :, :, 64:65], 1.0)

            # ---------------- helpers ----------------
            def emit_qk_group(aps, dst, wbv, bias, m, nb, tag="pqk",
                              act_evict=False):
                ps = aps.tile([P, QB], F32, tag=tag, name=f"pqk{m}_{nb}")
                yield  # allow interleave before the first MM
                for k in range(KC):
                    nc.tensor.matmul(
                        ps[:],
                        wbv[:, k, m * P:(m + 1) * P],
                        xq[:, nb, k, :],
                        start=(k == 0),
                        stop=(k == KC - 1),
                    )
                    yield
                dsl = dst[:, m * S + nb * QB: m * S + (nb + 1) * QB]
                if act_evict:
                    # ScalarE is idle during projections; its activation
                    # path does the bias-add eviction for free
                    nc.scalar.add(dsl, ps[:], bias[:, m:m + 1])
                else:
                    nc.vector.tensor_scalar_add(dsl, ps[:], bias[:, m:m + 1])

            def emit_v_group(aps, s, tag="pv"):
                ps = aps.tile([P, DHC], F32, tag=tag, name=f"pv{s}")
                yield
                for k in range(KC):
                    nc.tensor.matmul(
                        ps[:],
                        xq[:, s // 4, k, (s % 4) * P:(s % 4 + 1) * P],
                        wvv[:, k, :],
                        start=(k == 0),
                        stop=(k == KC - 1),
                    )
                    yield
                vout = Vb[:, s * 260:(s + 1) * 260].rearrange(
                    "p (h x) -> p h x", h=HPC)[:, :, 0:64]
                psr = ps[:].rearrange("p (h x) -> p h x", h=HPC)
                bvr = bv_bc[:].rearrange("p (h x) -> p h x", h=HPC)
                nc.vector.tensor_add(vout, psr, bvr)

            def run_interleaved(gens, pattern):
                """Round-robin generators following `pattern` (list of gen
                indices); a finished generator is skipped."""
                live = [iter(g) for g in gens]
                done = [False] * len(gens)
                pi = 0
                while not all(done):
                    g = pattern[pi % len(pattern)]
                    pi += 1
                    if done[g]:
                        continue
                    try:
                        next(live[g])
                    except StopIteration:
                        done[g] = True

            # ---------------- Phases A+B: projections merged into the
            # attention sweep. Head 0 runs jc ASCENDING, interleaved with
            # the just-in-time remainder of the projections (its natural
            # PE filler); heads 1-3 run jc DESCENDING with m=1 proj /
            # split output-projection fillers.
            with tc.tile_pool(name="opool", bufs=4, space="PSUM") as opool:
                def qk_m1_filler(dst, wbv, bias):
                    for nb in range(NQB):
                        yield from emit_qk_group(opool, dst, wbv, bias, 1, nb,
                                                 tag="oacc")

                def out0_filler(mo_range):
                    # c=0 half of the output projection (partial sums to
                    # outT0; host adds outT0+outT1) — PE filler during the
                    # ACT-bound attention phase. Output DMAs ride the idle
                    # GpSimd SWDGE ring.
                    for mo in mo_range:
                        ot = stage.tile([P, S], BF16, tag="outstage",
                                        name=f"ot0_{mo}")
                        for qb in range(NQB):
                            ps = opool.tile([P, QB], F32, tag="oacc",
                                            name=f"o0_{mo}_{qb}")
                            yield
                            nc.tensor.matmul(
                                ps[:],
                                wov[:, 0, mo * P:(mo + 1) * P],
                                OT[:, qb * QB:(qb + 1) * QB],
                                start=True, stop=True,
                            )
                            yield
                            nc.vector.tensor_copy(
                                ot[:, qb * QB:(qb + 1) * QB], ps[:])
                            yield
                        nc.gpsimd.dma_start(outT0[mo * P:(mo + 1) * P, :],
                                            ot[:])

                def norm_mul(h, m, poff, qb, drecr, o_q):
                    base = 64 if qb >= 2 else 0
                    rbp = opool.tile([64, QB], F32, tag="oacc",
                                     name=f"rbp{h}_{qb}")
                    nc.tensor.matmul(
                        rbp[:],
                        selb[base:base + 33, qb * 64:(qb + 1) * 64],
                        drecr[base:base + 33, :],
                        start=True, stop=True,
                    )
                    nc.vector.tensor_mul(
                        OT[poff:poff + 64,
                           m * S + qb * QB: m * S + (qb + 1) * QB],
                        o_q[:],
                        rbp[:],
                    )

                def head(h, scpool, ecw, asc=False, filler=None, drain=6,
                         pending_norm=None, split_norm=False):
                    m, poff = h // 2, 64 * (h % 2)
                    kt_h = KT[poff:poff + 64, m * S:(m + 1) * S]
                    qt_h = QT[poff:poff + 64, m * S:(m + 1) * S]
                    opsq = {}
                    oqd = {}
                    drec = small.tile([97, QB], F32, tag="drec", bufs=2,
                                      name=f"drec{h}")
                    drecr = small.tile([97, QB], BF16, tag="drecr", bufs=2,
                                       name=f"drecr{h}")
                    # unused rows must be finite for the batched reciprocal
                    nc.gpsimd.memset(drec[:], 1.0)
                    rng = range(SC) if asc else range(SC - 1, -1, -1)
                    for jc in rng:
                        if jc == SC - 3 and pending_norm is not None:
                            pending_norm()
                            pending_norm = None
                        # descending: filler competes for decay-freed O'
                        # psum slots (jc <= 11) after PV/norm demands;
                        # ascending: slots are free from the start
                        if filler is not None and (asc or jc <= SC - 5):
                            for _ in range(drain):
                                next(filler, None)
                        W = P * (jc + 1)
                        e = epool.tile([P, S], BF16, tag="e")
                        # scores S^T[jc] in chunked psum tiles; diagonal
                        # 128-block gets -1e30 masked via an extra
                        # accumulating identity x tri matmul
                        for c0 in range(0, W, ecw):
                            cw = min(ecw, W - c0)
                            ps = scpool.tile([P, ecw], F32, tag="sc")
                            for cc in range(0, cw, QB):
                                ccw = min(QB, cw - cc)
                                isdiag = (c0 + cc + ccw == W)
                                nc.tensor.matmul(
                                    ps[:, cc:cc + ccw],
                                    kt_h[:, jc * P:(jc + 1) * P],
                                    qt_h[:, c0 + cc:c0 + cc + ccw],
                                    start=True,
                                    stop=not isdiag,
                                )
                                if isdiag:
                                    d0 = W - P - c0
                                    nc.tensor.matmul(
                                        ps[:, d0:d0 + P],
                                        idb[:],
                                        trib[:],
                                        start=False,
                                        stop=True,
                                        skip_group_check=True,
                                    )
                            nc.scalar.activation(
                                e[:, c0:c0 + cw],
                                ps[:, 0:cw],
                                mybir.ActivationFunctionType.Exp,
                                scale=SCALE,
                            )
                        # PV accumulate; qb block b spans jc = 4b .. 15
                        for qb in range(jc // 4 + 1):
                            cw = min(QB, W - qb * QB)
                            pv_start = (jc == 4 * qb) if asc else (jc == SC - 1)
                            pv_stop = (jc == SC - 1) if asc else (jc == 4 * qb)
                            if pv_start:
                                opsq[qb] = opool.tile([65, QB], F32, tag="oacc",
                                                      name=f"oacc{h}_{qb}")
                            nc.tensor.matmul(
                                opsq[qb][:, 0:cw],
                                Vb[:, jc * 260 + 65 * h: jc * 260 + 65 * h + 65],
                                e[:, qb * QB:qb * QB + cw],
                                start=pv_start,
                                stop=pv_stop,
                            )
                            if pv_stop:
                                # this qb block is finished: stash the
                                # denominator row (to partition 32*qb) and
                                # the unnormalized O' block, free the slot
                                ops = opsq.pop(qb)
                                nc.vector.tensor_copy(
                                    drec[32 * qb:32 * qb + 1, :],
                                    ops[64:65, :])
                                o_q = small.tile([64, QB], BF16, tag="oq",
                                                 bufs=8, name=f"oq{h}_{qb}")
                                nc.vector.tensor_copy(o_q[:], ops[0:64, :])
                                oqd[qb] = o_q
                        if split_norm and jc == 8:
                            # qb3/qb2 denominators are final: normalize
                            # those blocks now so the tail's c=1 output
                            # projection can start with them immediately
                            with nc.allow_low_precision(reason="bf16 recip"):
                                nc.vector.reciprocal(drecr[64:97, :],
                                                     drec[64:97, :])
                            norm_mul(h, m, poff, 3, drecr, oqd[3])
                            norm_mul(h, m, poff, 2, drecr, oqd[2])
                    if filler is not None:
                        for _ in filler:
                            pass
                    if split_norm:
                        with nc.allow_low_precision(reason="bf16 recip"):
                            nc.vector.reciprocal(drecr[0:33, :], drec[0:33, :])
                        norm_mul(h, m, poff, 1, drecr, oqd[1])
                        norm_mul(h, m, poff, 0, drecr, oqd[0])
                        return None
                    # deferred: one batched reciprocal for the head's 4
                    # denominator rows, then per-block selector-broadcast +
                    # normalize (emitted inside the NEXT head so the PE
                    # stream doesn't stall behind the DVE reciprocal)
                    def norm(h=h, m=m, poff=poff, drec=drec, drecr=drecr,
                             oqd=oqd):
                        with nc.allow_low_precision(reason="bf16 recip"):
                            nc.vector.reciprocal(drecr[:], drec[:])
                        for qb in range(NQB - 1, -1, -1):
                            norm_mul(h, m, poff, qb, drecr, oqd[qb])
                    return norm

                with tc.tile_pool(name="scp0", bufs=4, space="PSUM") as scp0:
                    # pre-phase: V s0-3 + Q/K m0 nb0 (gates head0's jc 0-3)
                    pre = [emit_qk_group(opool, QT, wqv, bq_sb, 0, 0,
                                         tag="oacc", act_evict=True),
                           emit_qk_group(opool, KT, wkv, bk_sb, 0, 0,
                                         tag="oacc", act_evict=True)]
                    vs = [emit_v_group(opool, s, tag="oacc")
                          for s in range(4)]
                    run_interleaved(pre + vs, [0, 2, 2, 1, 3, 3,
                                               0, 4, 4, 1, 5, 5])
                    # just-in-time remainder: era q feeds head0's jc 4q+4..
                    units = []
                    for q in range(1, 4):
                        units.append(emit_qk_group(
                            opool, QT, wqv, bq_sb, 0, q, tag="oacc",
                            act_evict=True))
                        units.append(emit_v_group(opool, 4 * q, tag="oacc"))
                        units.append(emit_v_group(opool, 4 * q + 1, tag="oacc"))
                        units.append(emit_qk_group(
                            opool, KT, wkv, bk_sb, 0, q, tag="oacc",
                            act_evict=True))
                        units.append(emit_v_group(opool, 4 * q + 2, tag="oacc"))
                        units.append(emit_v_group(opool, 4 * q + 3, tag="oacc"))
                    proj_rest = itertools.chain(
                        *units, qk_m1_filler(QT, wqv, bq_sb))
                    n0 = head(0, scp0, QB, asc=True, filler=proj_rest,
                              drain=16)
                with tc.tile_pool(name="scp", bufs=2, space="PSUM") as scp:
                    n1 = head(1, scp, EC, filler=qk_m1_filler(KT, wkv, bk_sb),
                              pending_norm=n0)
                    n2 = head(2, scp, EC, filler=out0_filler(range(0, 4)),
                              pending_norm=n1)
                    head(3, scp, EC, filler=out0_filler(range(4, 8)),
                         pending_norm=n2, split_norm=True)

            # ---------------- Phase C: c=1 output projection half -------
            # qb-outer, descending: qb3/qb2 were normalized mid-head-3, so
            # their matmuls start without waiting for the final reciprocal
            with tc.tile_pool(name="cpsum", bufs=4, space="PSUM") as cps:
                for qb in range(NQB - 1, -1, -1):
                    for mo in range(D // P):
                        ps = cps.tile([P, QB], F32, tag="oproj")
                        nc.tensor.matmul(
                            ps[:],
                            wov[:, 1, mo * P:(mo + 1) * P],
                            OT[:, S + qb * QB: S + (qb + 1) * QB],
                            start=True, stop=True,
                        )
                        ot = small.tile([P, QB], BF16, tag="o1s", bufs=8,
                                        name=f"o1s_{mo}_{qb}")
                        if mo % 2 == 0:
                            nc.scalar.copy(ot[:], ps[:])
                        else:
                            nc.vector.tensor_copy(ot[:], ps[:])
                        nc.gpsimd.dma_start(
                            outT1[mo * P:(mo + 1) * P,
                                  qb * QB:(qb + 1) * QB], ot[:])

    _legalize_waits(nc)
    return nc


def _get_nc():
    global _COMPILED
    if _COMPILED is None:
        _COMPILED = _build_nc()
    return _COMPILED


def _make_in_maps(x, wq, bq, wk, bk, wv, bv, wo, bo):
    import ml_dtypes
    bf16 = ml_dtypes.bfloat16
    k = np.arange(P)
    trin = np.where(k[:, None] >= k[None, :], 0.0, -1e30).astype(bf16)
    iden = np.eye(P, dtype=bf16)
    sel = np.zeros((97, 256), dtype=bf16)
    for r in range(4):
        sel[32 * r, r * 64:(r + 1) * 64] = 1.0
    def pack_w(w):
        # [1024, C] -> [128, (k, C)] : row k*128+p lands at (p, k*C+c)
        C = w.shape[1]
        return np.ascontiguousarray(
            w.astype(bf16).reshape(KC, P, C).transpose(1, 0, 2).reshape(P, KC * C))

    xTs = []
    for b in range(B):
        # x^T [1024, 2048] -> [128, (qd, k, s')]
        xt = np.ascontiguousarray(x[b].T).astype(bf16)
        xTs.append(np.ascontiguousarray(
            xt.reshape(KC, P, 4, 512).transpose(1, 2, 0, 3).reshape(P, KC * S)))
    in_maps = []
    for c in range(NCORES):
        b, g = c // 4, c % 4
        cols = slice(DHC * g, DHC * (g + 1))
        wo_g = wo[cols, :]  # [256, 1024]
        wo_p = np.ascontiguousarray(
            wo_g.astype(bf16).reshape(2, P, D).transpose(1, 0, 2).reshape(P, 2 * D))
        in_maps.append({
            "xT": xTs[b],
            "wq": pack_w(np.asarray(wq[:, cols])),
            "wk": pack_w(np.asarray(wk[:, cols])),
            "wv": pack_w(np.asarray(wv[:, cols])),
            "wo": wo_p,
            "bq": np.ascontiguousarray(bq[cols]).reshape(2, P, 1),
            "bk": np.ascontiguousarray(bk[cols]).reshape(2, P, 1),
            "bv": np.ascontiguousarray(np.broadcast_to(bv[cols].reshape(1, DHC), (P, DHC))),
            "trin": trin,
            "iden": iden,
            "sel": sel,
        })
    return in_maps


def kernel(x, wq, bq, wk, bk, wv, bv, wo, bo, _trace=False, _trace_kwargs=None):
    x = np.asarray(x, dtype=np.float32)
    assert x.shape == (B, S, D), x.shape
    nc = _get_nc()
    in_maps = _make_in_maps(
        x, np.asarray(wq), np.asarray(bq), np.asarray(wk), np.asarray(bk),
        np.asarray(wv), np.asarray(bv), np.asarray(wo), np.asarray(bo))
    kw = {}
    if _trace:
        kw = dict(trace=True, **(_trace_kwargs or {}))
    res = run_bass_kernel_spmd(nc, in_maps, list(range(NCORES)), **kw)
    out = np.empty((B, S, D), dtype=np.float32)
    for b in range(B):
        acc = np.zeros((D, S), dtype=np.float64)
        for g in range(4):
            acc += res.results[4 * b + g]["outT0"].astype(np.float64)
            acc += res.results[4 * b + g]["outT1"].astype(np.float64)
        out[b] = acc.T.astype(np.float32) + np.asarray(bo, dtype=np.float32)
    kernel.last_result = res
    return out


# revision 56
# speedup vs baseline: 1.1685x; 1.0237x over previous
"""Trainium2 Bass kernel for 16-head self-attention (D=1024, S=2048, B=2)
with upper-triangular (j >= i) mask and scale 1/head_dim.

Sharding: batch*head-group parallel over 8 cores. Core c handles batch
c//4, heads [4*(c%4), 4*(c%4)+4). Each core computes Q/K/V projections for
its 256 output dims, attention for its 4 heads, and a partial output
projection (its 256 rows of wo). Host sums the 4 partials per batch.

On-chip layout is transposed end-to-end: QT/KT [dh, seq], scores S^T
[seq_k, seq_q] (stationary=K^T chunk, moving=Q^T), exp on ScalarE
PSUM->SBUF with the 1/64 scale folded in, PV as O'^T = V'^T E^T with a
ones-column appended to V so row 64 of O' is the softmax denominator,
then out^T = wo^T O^T (bf16 partials). The host transposes back.

Perf structure (v2):
- mask applied inside the score PSUM accumulation via an extra
  identity x (-1e30 upper-tri) matmul -> no post-exp mask stage.
- attention iterates key chunks jc DESCENDING so the per-qb O' PSUM
  banks free progressively; each qb block normalizes early with
  reciprocal_approx_fast + a K=1 ones matmul broadcast.
- exp runs on [128, 1024] two-bank PSUM chunks (fewer ACTIVATEs).
- Q/K (m=0) and V projections interleave 1:2 so V's per-matmul
  LDWEIGHTS hides under the 512-wide Q/K matmuls; m=1 projections are
  emitted between attention heads as PE filler (and reuse freed O'
  PSUM slots), keeping the PE dense enough to hold the HAM clock warm.
- batched input DMAs (one per weight tensor, x in 4 column quarters).
"""

import itertools
import sys

sys.path.insert(0, "/opt/trn_rl_repo")

import numpy as np

import concourse.bass as bass
import concourse.mybir as mybir
from concourse import tile
from concourse.bass_utils import run_bass_kernel_spmd

# ---------------------------------------------------------------------------
# Workaround: this walrus build supports only 1 sync wait on the SP CTRL
# (drain) instruction; split the TileContext exit drain's waits across
# sequential drains (same-engine program order makes this equivalent).
_MAX_DRAIN_WAITS = 1


def _patched_drain_and_barrier(self, tick_clock, wait_clock):
    from bass_rust import ScopedClock

    nc = self.nc
    drain_inst = nc.sync.drain()
    wait_clock.add_sem_waits(
        drain_inst.ins, ScopedClock({None: tick_clock.global_clock})
    )
    si = drain_inst.ins.sync_info
    if si is not None and len(si.on_wait) > _MAX_DRAIN_WAITS:
        waits = list(si.on_wait)
        si.on_wait = waits[:_MAX_DRAIN_WAITS]
        rest = waits[_MAX_DRAIN_WAITS:]
        while rest:
            chunk, rest = rest[:_MAX_DRAIN_WAITS], rest[_MAX_DRAIN_WAITS:]
            extra = nc.sync.drain()
            esi = extra.ins.sync_info
            if esi is None:
                extra.ins.sync_info = mybir.SyncInfo(on_wait=chunk, on_update=[])
            else:
                esi.on_wait = chunk
    nc.all_engine_barrier()
    assert self.sems is not None
    popped = nc._tile_sem_poison_stack.pop()
    assert popped is self._sem_poison
    nc.clear_and_free_semaphores(list(self.sems.allocated().values()))
    nc.all_engine_barrier()


tile.TileContext._drain_and_barrier = _patched_drain_and_barrier


def _legalize_waits(nc, max_waits=1):
    """This walrus build accepts at most one sync wait per instruction.
    Hoist extra waits onto preceding NoOps on the same engine (same-engine
    program order preserves the gating semantics)."""
    for blk in nc.main_func.blocks:
        out = []
        for inst in blk.instructions:
            si = inst.sync_info
            if si is not None and len(si.on_wait) > max_waits:
                waits = list(si.on_wait)
                si.on_wait = waits[-max_waits:]
                for w in waits[:-max_waits]:
                    nop = mybir.InstNoOp(
                        name=nc.get_next_instruction_name(), ins=[], outs=[]
                    )
                    nop.engine = inst.engine
                    nop.sync_info = mybir.SyncInfo(on_wait=[w], on_update=[])
                    nc.register_instruction(nop)
                    out.append(nop)
            out.append(inst)
        blk.instructions[:] = out


# ---------------------------------------------------------------------------

B, S, D = 2, 2048, 1024
H, HD = 16, 64
SCALE = 1.0 / HD
NCORES = 8
HPC = 4          # heads per core
DHC = HPC * HD   # 256 head-dims per core
P = 128
KC = D // P      # 8 contraction chunks for projections
SC = S // P      # 16 seq chunks of 128
QB = 512         # seq_q block for PV / O-proj
NQB = S // QB    # 4
EC = 1024        # exp chunk width (2 PSUM banks)

F32 = mybir.dt.float32
F32R = mybir.dt.float32r
BF16 = mybir.dt.bfloat16

_COMPILED = None


def _build_nc():
    nc = bass.Bass("TRN2", target_bir_lowering=False, debug=False,
                   num_devices=NCORES)

    # All big inputs are pre-packed host-side into the exact [128, N]
    # SBUF layouts so every load is a contiguous max-rate 2D DMA.
    # xT packed as [p, (quarter, k, s')]: col = qd*4096 + k*512 + s'
    xT = nc.declare_dram_parameter("xT", [P, KC * S], BF16, isOutput=False)
    wq = nc.declare_dram_parameter("wq", [P, KC * DHC], BF16, isOutput=False)
    wk = nc.declare_dram_parameter("wk", [P, KC * DHC], BF16, isOutput=False)
    wv = nc.declare_dram_parameter("wv", [P, KC * DHC], BF16, isOutput=False)
    wo = nc.declare_dram_parameter("wo", [P, 2 * D], BF16, isOutput=False)
    bq = nc.declare_dram_parameter("bq", [2, P, 1], F32, isOutput=False)
    bk = nc.declare_dram_parameter("bk", [2, P, 1], F32, isOutput=False)
    bv = nc.declare_dram_parameter("bv", [P, DHC], F32, isOutput=False)
    trin = nc.declare_dram_parameter("trin", [P, P], BF16, isOutput=False)
    iden = nc.declare_dram_parameter("iden", [P, P], BF16, isOutput=False)
    sel = nc.declare_dram_parameter("sel", [97, 4 * 64], BF16, isOutput=False)
    outT0 = nc.declare_dram_parameter("outT0", [D, S], BF16, isOutput=True)
    outT1 = nc.declare_dram_parameter("outT1", [D, S], BF16, isOutput=True)

    with tile.TileContext(nc) as tc:
        dmaq = [nc.sync, nc.scalar]
        dq = [0]

        def dma(out_ap, in_ap):
            eng = dmaq[dq[0] % len(dmaq)]
            dq[0] += 1
            return eng.dma_start(out_ap, in_ap)

        with (
            tc.tile_pool(name="persist", bufs=1) as pp,
            tc.tile_pool(name="stage", bufs=2) as stage,
            tc.tile_pool(name="epool", bufs=6) as epool,
            tc.tile_pool(name="small", bufs=4) as small,
        ):
            # ---------------- persistent SBUF tensors ----------------
            xb = pp.tile([P, KC * S], BF16, tag="xb")          # 32KB
            wqb = pp.tile([P, KC * DHC], BF16, tag="wqb")      # 4KB
            wkb = pp.tile([P, KC * DHC], BF16, tag="wkb")
            wvb = pp.tile([P, KC * DHC], BF16, tag="wvb")
            wob = pp.tile([P, 2 * D], BF16, tag="wob")
            QT = pp.tile([P, 2 * S], BF16, tag="qt")           # 8KB
            KT = pp.tile([P, 2 * S], BF16, tag="kt")
            # V with a ones column per head: 16 chunks x [h0(64) 1 | ...]
            Vb = pp.tile([P, SC * HPC * 65], BF16, tag="vb")   # 8.3KB
            OT = pp.tile([P, 2 * S], BF16, tag="ot")
            trib = pp.tile([P, P], BF16, tag="trib")
            idb = pp.tile([P, P], BF16, tag="idb")
            bq_sb = pp.tile([P, 2], F32, tag="bq")
            bk_sb = pp.tile([P, 2], F32, tag="bk")
            bv_bc = pp.tile([P, DHC], F32, tag="bvbc")
            # One-hot selector: sel[:, 64r:64r+64] has row 32r all-ones
            # (denominator rows sit at 32-aligned partitions; the K=97
            # matmul broadcasts row 32r of drecr to 64 partitions).
            selb = pp.tile([97, 4 * 64], BF16, tag="selb")

            # xb layout [p, (qd, k, s')]: QK group nb reads
            # xq[:, nb, k, :]; V s-chunk reads xq[:, s//4, k, (s%4)*128..]
            xq = xb[:].rearrange("p (q k s) -> p q k s", q=4, k=KC)
            wqv = wqb[:].rearrange("p (k c) -> p k c", k=KC)
            wkv = wkb[:].rearrange("p (k c) -> p k c", k=KC)
            wvv = wvb[:].rearrange("p (k c) -> p k c", k=KC)
            wov = wob[:].rearrange("p (c d) -> p c d", c=2)

            # ---------------- input DMAs (all contiguous 2D) ----------
            nc.sync.dma_start(bq_sb[:, 0:1], bq[0])
            nc.sync.dma_start(bq_sb[:, 1:2], bq[1])
            nc.scalar.dma_start(bk_sb[:, 0:1], bk[0])
            nc.scalar.dma_start(bk_sb[:, 1:2], bk[1])
            # sync ring: x quarters (the critical path); scalar: weights
            for qd in range(4):
                nc.sync.dma_start(xb[:, qd * 4096:(qd + 1) * 4096],
                                  xT[:, qd * 4096:(qd + 1) * 4096])
            nc.scalar.dma_start(wqb[:], wq[:, :])
            nc.scalar.dma_start(wvb[:], wv[:, :])
            nc.scalar.dma_start(wkb[:], wk[:, :])
            nc.scalar.dma_start(bv_bc[:], bv[:, :])
            nc.scalar.dma_start(trib[:], trin[:, :])
            nc.scalar.dma_start(idb[:], iden[:, :])
            nc.scalar.dma_start(wob[:], wo[:, :])
            nc.scalar.dma_start(selb[:], sel[:, :])
            # ones column of V (col 64 of each head's 65-block)
            nc.gpsimd.memset(
                Vb[:].rearrange("p (s h x) -> p s h x", s=SC, h=HPC)
                [:, :, :, 64:65], 1.0)

            # ---------------- helpers ----------------
            def emit_qk_group(aps, dst, wbv, bias, m, nb, tag="pqk",
                              act_evict=False):
                ps = aps.tile([P, QB], F32, tag=tag, name=f"pqk{m}_{nb}")
                yield  # allow interleave before the first MM
                for k in range(KC):
                    nc.tensor.matmul(
                        ps[:],
                        wbv[:, k, m * P:(m + 1) * P],
                        xq[:, nb, k, :],
                        start=(k == 0),
                        stop=(k == KC - 1),
                    )
                    yield
                dsl = dst[:, m * S + nb * QB: m * S + (nb + 1) * QB]
                if act_evict:
                    # ScalarE is idle during projections; its activation
                    # path does the bias-add eviction for free
                    nc.scalar.add(dsl, ps[:], bias[:, m:m + 1])
                else:
                    nc.vector.tensor_scalar_add(dsl, ps[:], bias[:, m:m + 1])

            def emit_v_group(aps, s, tag="pv"):
                ps = aps.tile([P, DHC], F32, tag=tag, name=f"pv{s}")
                yield
                for k in range(KC):
                    nc.tensor.matmul(
                        ps[:],
                        xq[:, s // 4, k, (s % 4) * P:(s % 4 + 1) * P],
                        wvv[:, k, :],
                        start=(k == 0),
                        stop=(k == KC - 1),
                    )
                    yield
                vout = Vb[:, s * 260:(s + 1) * 260].rearrange(
                    "p (h x) -> p h x", h=HPC)[:, :, 0:64]
                psr = ps[:].rearrange("p (h x) -> p h x", h=HPC)
                bvr = bv_bc[:].rearrange("p (h x) -> p h x", h=HPC)
                nc.vector.tensor_add(vout, psr, bvr)

            def run_interleaved(gens, pattern):
                """Round-robin generators following `pattern` (list of gen
                indices); a finished generator is skipped."""
                live = [iter(g) for g in gens]
                done = [False] * len(gens)
                pi = 0
                while not all(done):
                    g = pattern[pi % len(pattern)]
                    pi += 1
                    if done[g]:
                        continue
                    try:
                        next(live[g])
                    except StopIteration:
                        done[g] = True

            # ---------------- Phases A+B: projections merged into the
            # attention sweep. Head 0 runs jc ASCENDING, interleaved with
            # the just-in-time remainder of the projections (its natural
            # PE filler); heads 1-3 run jc DESCENDING with m=1 proj /
            # split output-projection fillers.
            with tc.tile_pool(name="opool", bufs=4, space="PSUM") as opool:
                def qk_m1_filler(dst, wbv, bias):
                    for nb in range(NQB):
                        yield from emit_qk_group(opool, dst, wbv, bias, 1, nb,
                                                 tag="oacc")

                def out0_filler(mo_range):
                    # c=0 half of the output projection (partial sums to
                    # outT0; host adds outT0+outT1) — PE filler during the
                    # ACT-bound attention phase. Output DMAs ride the idle
                    # GpSimd SWDGE ring.
                    for mo in mo_range:
                        ot = stage.tile([P, S], BF16, tag="outstage",
                                        name=f"ot0_{mo}")
                        for qb in range(NQB):
                            ps = opool.tile([P, QB], F32, tag="oacc",
                                            name=f"o0_{mo}_{qb}")
                            yield
                            nc.tensor.matmul(
                                ps[:],
                                wov[:, 0, mo * P:(mo + 1) * P],
                                OT[:, qb * QB:(qb + 1) * QB],
                                start=True, stop=True,
                            )
                            yield
                            nc.vector.tensor_copy(
                                ot[:, qb * QB:(qb + 1) * QB], ps[:])
                            yield
                        nc.gpsimd.dma_start(outT0[mo * P:(mo + 1) * P, :],
                                            ot[:])

                def norm_mul(h, m, poff, qb, drecr, o_q):
                    base = 64 if qb >= 2 else 0
                    rbp = opool.tile([64, QB], F32, tag="oacc",
                                     name=f"rbp{h}_{qb}")
                    nc.tensor.matmul(
                        rbp[:],
                        selb[base:base + 33, qb * 64:(qb + 1) * 64],
                        drecr[base:base + 33, :],
                        start=True, stop=True,
                    )
                    nc.vector.tensor_mul(
                        OT[poff:poff + 64,
                           m * S + qb * QB: m * S + (qb + 1) * QB],
                        o_q[:],
                        rbp[:],
                    )

                def head(h, scpool, ecw, asc=False, filler=None, drain=8,
                         pending_norm=None, split_norm=False):
                    m, poff = h // 2, 64 * (h % 2)
                    kt_h = KT[poff:poff + 64, m * S:(m + 1) * S]
                    qt_h = QT[poff:poff + 64, m * S:(m + 1) * S]
                    opsq = {}
                    oqd = {}
                    drec = small.tile([97, QB], F32, tag="drec", bufs=2,
                                      name=f"drec{h}")
                    drecr = small.tile([97, QB], BF16, tag="drecr", bufs=2,
                                       name=f"drecr{h}")
                    # unused rows must be finite for the batched reciprocal
                    nc.gpsimd.memset(drec[:], 1.0)
                    rng = range(SC) if asc else range(SC - 1, -1, -1)
                    for jc in rng:
                        if jc == SC - 3 and pending_norm is not None:
                            pending_norm()
                            pending_norm = None
                        # descending: filler competes for decay-freed O'
                        # psum slots (jc <= 11) after PV/norm demands;
                        # ascending: slots are free from the start
                        if filler is not None and (asc or jc <= SC - 5):
                            for _ in range(drain):
                                next(filler, None)
                        W = P * (jc + 1)
                        e = epool.tile([P, S], BF16, tag="e")
                        # scores S^T[jc] in chunked psum tiles; diagonal
                        # 128-block gets -1e30 masked via an extra
                        # accumulating identity x tri matmul
                        for c0 in range(0, W, ecw):
                            cw = min(ecw, W - c0)
                            ps = scpool.tile([P, ecw], F32, tag="sc")
                            for cc in range(0, cw, QB):
                                ccw = min(QB, cw - cc)
                                isdiag = (c0 + cc + ccw == W)
                                nc.tensor.matmul(
                                    ps[:, cc:cc + ccw],
                                    kt_h[:, jc * P:(jc + 1) * P],
                                    qt_h[:, c0 + cc:c0 + cc + ccw],
                                    start=True,
                                    stop=not isdiag,
                                )
                                if isdiag:
                                    d0 = W - P - c0
                                    nc.tensor.matmul(
                                        ps[:, d0:d0 + P],
                                        idb[:],
                                        trib[:],
                                        start=False,
                                        stop=True,
                                        skip_group_check=True,
                                    )
                            nc.scalar.activation(
                                e[:, c0:c0 + cw],
                                ps[:, 0:cw],
                                mybir.ActivationFunctionType.Exp,
                                scale=SCALE,
                            )
                        # PV accumulate; qb block b spans jc = 4b .. 15
                        for qb in range(jc // 4 + 1):
                            cw = min(QB, W - qb * QB)
                            pv_start = (jc == 4 * qb) if asc else (jc == SC - 1)
                            pv_stop = (jc == SC - 1) if asc else (jc == 4 * qb)
                            if pv_start:
                                opsq[qb] = opool.tile([65, QB], F32, tag="oacc",
                                                      name=f"oacc{h}_{qb}")
                            nc.tensor.matmul(
                                opsq[qb][:, 0:cw],
                                Vb[:, jc * 260 + 65 * h: jc * 260 + 65 * h + 65],
                                e[:, qb * QB:qb * QB + cw],
                                start=pv_start,
                                stop=pv_stop,
                            )
                            if pv_stop:
                                # this qb block is finished: stash the
                                # denominator row (to partition 32*qb) and
                                # the unnormalized O' block, free the slot
                                ops = opsq.pop(qb)
                                nc.vector.tensor_copy(
                                    drec[32 * qb:32 * qb + 1, :],
                                    ops[64:65, :])
                                o_q = small.tile([64, QB], BF16, tag="oq",
                                                 bufs=10, name=f"oq{h}_{qb}")
                                nc.vector.tensor_copy(o_q[:], ops[0:64, :])
                                oqd[qb] = o_q
                        if split_norm and jc == 8:
                            # qb3/qb2 denominators are final: normalize
                            # those blocks now so the tail's c=1 output
                            # projection can start with them immediately
                            with nc.allow_low_precision(reason="bf16 recip"):
                                nc.vector.reciprocal(drecr[64:97, :],
                                                     drec[64:97, :])
                            norm_mul(h, m, poff, 3, drecr, oqd[3])
                            norm_mul(h, m, poff, 2, drecr, oqd[2])
                    if filler is not None:
                        for _ in filler:
                            pass
                    if split_norm:
                        with nc.allow_low_precision(reason="bf16 recip"):
                            nc.vector.reciprocal(drecr[0:33, :], drec[0:33, :])
                        norm_mul(h, m, poff, 1, drecr, oqd[1])
                        norm_mul(h, m, poff, 0, drecr, oqd[0])
                        return None
                    # deferred: one batched reciprocal for the head's 4
                    # denominator rows, then per-block selector-broadcast +
                    # normalize (emitted inside the NEXT head so the PE
                    # stream doesn't stall behind the DVE reciprocal)
                    def norm(h=h, m=m, poff=poff, drec=drec, drecr=drecr,
                             oqd=oqd):
                        with nc.allow_low_precision(reason="bf16 recip"):
                            nc.vector.reciprocal(drecr[:], drec[:])
                        for qb in range(NQB - 1, -1, -1):
                            norm_mul(h, m, poff, qb, drecr, oqd[qb])
                    return norm

                with tc.tile_pool(name="scp0", bufs=4, space="PSUM") as scp0:
                    # pre-phase: V s0-3 + Q/K m0 nb0 (gates head0's jc 0-3)
                    pre = [emit_qk_group(opool, QT, wqv, bq_sb, 0, 0,
                                         tag="oacc", act_evict=True),
                           emit_qk_group(opool, KT, wkv, bk_sb, 0, 0,
                                         tag="oacc", act_evict=True)]
                    vs = [emit_v_group(opool, s, tag="oacc")
                          for s in range(4)]
                    run_interleaved(pre + vs, [0, 2, 2, 1, 3, 3,
                                               0, 4, 4, 1, 5, 5])
                    # just-in-time remainder: era q feeds head0's jc 4q+4..
                    units = []
                    for q in range(1, 4):
                        units.append(emit_qk_group(
                            opool, QT, wqv, bq_sb, 0, q, tag="oacc",
                            act_evict=True))
                        units.append(emit_v_group(opool, 4 * q, tag="oacc"))
                        units.append(emit_v_group(opool, 4 * q + 1, tag="oacc"))
                        units.append(emit_qk_group(
                            opool, KT, wkv, bk_sb, 0, q, tag="oacc",
                            act_evict=True))
                        units.append(emit_v_group(opool, 4 * q + 2, tag="oacc"))
                        units.append(emit_v_group(opool, 4 * q + 3, tag="oacc"))
                    proj_rest = itertools.chain(
                        *units, qk_m1_filler(QT, wqv, bq_sb))
                    n0 = head(0, scp0, QB, asc=True, filler=proj_rest,
                              drain=16)
                with tc.tile_pool(name="scp", bufs=2, space="PSUM") as scp:
                    n1 = head(1, scp, EC, filler=qk_m1_filler(KT, wkv, bk_sb),
                              pending_norm=n0)
                    n2 = head(2, scp, EC, filler=out0_filler(range(0, 4)),
                              pending_norm=n1)
                    head(3, scp, EC, filler=out0_filler(range(4, 8)),
                         pending_norm=n2, split_norm=True)

            # ---------------- Phase C: c=1 output projection half -------
            # qb-outer, descending: qb3/qb2 were normalized mid-head-3, so
            # their matmuls start without waiting for the final reciprocal
            with tc.tile_pool(name="cpsum", bufs=4, space="PSUM") as cps:
                for qb in range(NQB - 1, -1, -1):
                    for mo in range(D // P):
                        ps = cps.tile([P, QB], F32, tag="oproj")
                        nc.tensor.matmul(
                            ps[:],
                            wov[:, 1, mo * P:(mo + 1) * P],
                            OT[:, S + qb * QB: S + (qb + 1) * QB],
                            start=True, stop=True,
                        )
                        ot = small.tile([P, QB], BF16, tag="o1s", bufs=8,
                                        name=f"o1s_{mo}_{qb}")
                        if mo % 2 == 0:
                            nc.scalar.copy(ot[:], ps[:])
                        else:
                            nc.vector.tensor_copy(ot[:], ps[:])
                        nc.gpsimd.dma_start(
                            outT1[mo * P:(mo + 1) * P,
                                  qb * QB:(qb + 1) * QB], ot[:])

    _legalize_waits(nc)
    return nc


def _get_nc():
    global _COMPILED
    if _COMPILED is None:
        _COMPILED = _build_nc()
    return _COMPILED


def _make_in_maps(x, wq, bq, wk, bk, wv, bv, wo, bo):
    import ml_dtypes
    bf16 = ml_dtypes.bfloat16
    k = np.arange(P)
    trin = np.where(k[:, None] >= k[None, :], 0.0, -1e30).astype(bf16)
    iden = np.eye(P, dtype=bf16)
    sel = np.zeros((97, 256), dtype=bf16)
    for r in range(4):
        sel[32 * r, r * 64:(r + 1) * 64] = 1.0
    def pack_w(w):
        # [1024, C] -> [128, (k, C)] : row k*128+p lands at (p, k*C+c)
        C = w.shape[1]
        return np.ascontiguousarray(
            w.astype(bf16).reshape(KC, P, C).transpose(1, 0, 2).reshape(P, KC * C))

    xTs = []
    for b in range(B):
        # x^T [1024, 2048] -> [128, (qd, k, s')]
        xt = np.ascontiguousarray(x[b].T).astype(bf16)
        xTs.append(np.ascontiguousarray(
            xt.reshape(KC, P, 4, 512).transpose(1, 2, 0, 3).reshape(P, KC * S)))
    in_maps = []
    for c in range(NCORES):
        b, g = c // 4, c % 4
        cols = slice(DHC * g, DHC * (g + 1))
        wo_g = wo[cols, :]  # [256, 1024]
        wo_p = np.ascontiguousarray(
            wo_g.astype(bf16).reshape(2, P, D).transpose(1, 0, 2).reshape(P, 2 * D))
        in_maps.append({
            "xT": xTs[b],
            "wq": pack_w(np.asarray(wq[:, cols])),
            "wk": pack_w(np.asarray(wk[:, cols])),
            "wv": pack_w(np.asarray(wv[:, cols])),
            "wo": wo_p,
            "bq": np.ascontiguousarray(bq[cols]).reshape(2, P, 1),
            "bk": np.ascontiguousarray(bk[cols]).reshape(2, P, 1),
            "bv": np.ascontiguousarray(np.broadcast_to(bv[cols].reshape(1, DHC), (P, DHC))),
            "trin": trin,
            "iden": iden,
            "sel": sel,
        })
    return in_maps


def kernel(x, wq, bq, wk, bk, wv, bv, wo, bo, _trace=False, _trace_kwargs=None):
    x = np.asarray(x, dtype=np.float32)
    assert x.shape == (B, S, D), x.shape
    nc = _get_nc()
    in_maps = _make_in_maps(
        x, np.asarray(wq), np.asarray(bq), np.asarray(wk), np.asarray(bk),
        np.asarray(wv), np.asarray(bv), np.asarray(wo), np.asarray(bo))
    kw = {}
    if _trace:
        kw = dict(trace=True, **(_trace_kwargs or {}))
    res = run_bass_kernel_spmd(nc, in_maps, list(range(NCORES)), **kw)
    out = np.empty((B, S, D), dtype=np.float32)
    for b in range(B):
        acc = np.zeros((D, S), dtype=np.float64)
        for g in range(4):
            acc += res.results[4 * b + g]["outT0"].astype(np.float64)
            acc += res.results[4 * b + g]["outT1"].astype(np.float64)
        out[b] = acc.T.astype(np.float32) + np.asarray(bo, dtype=np.float32)
    kernel.last_result = res
    return out
